# revision 3
# baseline (speedup 1.0000x reference)
"""MCCDecoderAttention Trainium2 kernel (8 NeuronCores), v3.

Sharding: core = b*4 + g  (b in {0,1} batch, g in {0..3} head-group).
Each core computes attention for 3 heads of one batch plus its partial
output projection; the host sums 4 partials per batch and adds b_proj.

Layout (all matmul operands bf16, f32 PSUM accumulation):
  - Q/K projected feature-major into packed 128-row tiles
    qk0=[q0;q1], qk1=[k0;k1], qk2=[q2;k2]; q2 is copied to partitions
    64:128 of qk2b by an SBUF->SBUF DMA so head 2's matmuls have
    matching base partitions.
  - V projected token-major straight into vsb[key, head, 0:64].
  - Scores S^T computed per (head, 1024-query chunk, 128-key tile);
    exp on ScalarE (scale=1/8 folded, no max subtraction needed).
  - AV uses the transposed form: lhsT = A^T subtile [128k x 128q],
    rhs = vsb [128k x 64] -> psum [128q, 64] accumulated over key
    tiles; the softmax denominator L accumulates in parallel via
    1-column ones matmuls.  The narrow free dims halve PE cost vs the
    feature-major AV form.
  - Decoder mask: keys limited to [0, N-u); each unseen query gets its
    diagonal term back via a 1-column ones-matmul of q*k (elementwise),
    exp, and a fused DVE multiply-add into the psum accumulator.
  - Normalized ao is written token-major f32 with heads 0/1 interleaved
    per query tile; PE transposes [128q x 128c] tiles (into spare
    columns of the L psum bank) flip it feature-major for the output
    projection, which contracts [128ci]+[64ci] per psum group.

Scheduling: emission is software-pipelined (ST(t) is emitted before the
AV block of t-1 so the PE never queues behind the exp dependency), and
all projection / transpose / phase-3 work is drained as per-key-tile
"fillers" inside the Act-bound attention loops.
"""

import functools
import os
import sys

for _p in ("/opt/trn_rl_repo", "/root/.axon_site/_ro/trn_rl_repo"):
    if os.path.isdir(_p) and _p not in sys.path:
        sys.path.insert(0, _p)

import numpy as np

import concourse.bacc as bacc
import concourse.tile as tile
from concourse import mybir

N, C, D = 2048, 768, 64
NH = 3            # heads per core
CT = C // 128     # 6 contraction tiles
F32 = mybir.dt.float32
BF16 = mybir.dt.bfloat16
FP8 = mybir.dt.float8e4
DR = mybir.MatmulPerfMode.DoubleRow
NPBF = mybir.dt.np(BF16)
EXP = mybir.ActivationFunctionType.Exp
CPY = mybir.ActivationFunctionType.Copy
MUL = mybir.AluOpType.mult
ADD = mybir.AluOpType.add

_last_results = None  # BassKernelResults of the most recent run (for test.py)


@functools.lru_cache(maxsize=4)
def _build(u: int):
    assert u % 128 == 0 and 0 <= u <= 512, f"unsupported unseen_size {u}"
    nc = bacc.Bacc(None, target_bir_lowering=False)
    xT = nc.dram_tensor("xT", [C, N], BF16, kind="ExternalInput")
    wqkT = nc.dram_tensor("wqkT", [C, 6 * D], BF16, kind="ExternalInput")
    wvT = nc.dram_tensor("wvT", [C, NH * D], BF16, kind="ExternalInput")
    wpT = nc.dram_tensor("wpT", [NH * D, C], BF16, kind="ExternalInput")
    yT = nc.dram_tensor("yT", [C, N], BF16, kind="ExternalOutput")

    kfull = N - u
    T = (kfull + 127) // 128           # key tiles
    QT = N // 128                      # 16 query tiles
    ut0 = kfull // 128                 # first unseen query tile
    NCH = N // 512                     # 4 token chunks
    NU = u // 128                      # unseen tiles

    with nc.allow_low_precision(reason="bf16 staging/outputs"), \
         tile.TileContext(nc) as tc:
        with tc.tile_pool(name="persist", bufs=1) as P:
            xt = P.tile([128, CT, N], BF16)
            wqk = P.tile([128, CT, 6 * D], BF16)
            wv = P.tile([128, CT, NH * D], BF16)
            wp01 = P.tile([128, C], BF16)
            wp2 = P.tile([64, C], BF16)
            qk0 = P.tile([128, N], BF16)   # [q0; q1]
            qk1 = P.tile([128, N], BF16)   # [k0; k1]
            qk2 = P.tile([128, N], BF16)   # [q2; k2]
            qk2b = P.tile([128, N], BF16)  # [-; q2] (dma partition shift)
            vsb = P.tile([128, QT, NH, 64], BF16)
            # token-major normalized attention out: heads 0/1 interleaved per
            # query tile; head 2 in the low half of its own tile (high half
            # junk, transposed but never copied out).
            ao01 = P.tile([128, QT, 128], F32)
            ao2 = P.tile([128, QT, 128], F32)
            aoT01 = P.tile([128, N], BF16)
            aoT2 = P.tile([64, N], BF16)
            otile = [P.tile([128, N], BF16, name=f"ot{co}", tag=f"ot{co}")
                     for co in range(CT)]
            ones1 = P.tile([128, 1], BF16)
            ident = P.tile([128, 128], F32)
            idones = P.tile([128, 128], F32)
            prod = [P.tile([128, max(u, 1)], BF16, name=f"pr{h}", tag=f"pr{h}")
                    for h in range(NH)] if u else []
            esb = [P.tile([128, max(NU, 1)], F32, name=f"e{h}", tag=f"e{h}")
                   for h in range(NH)] if u else []

            nc.vector.memset(ones1[:], 1.0)
            nc.vector.memset(idones[:], 1.0)
            nc.gpsimd.affine_select(
                ident[:], idones[:], pattern=[[-1, 128]],
                compare_op=mybir.AluOpType.is_equal, fill=0.0,
                base=0, channel_multiplier=1)

            # loads: SP queue carries the QK critical path (wqk cols 0:256
            # then x chunks 0,1,3); the idle Act queue takes wv + x chunk 2.
            def load_x(ch, eng=None):
                sl = slice(ch * 512, (ch + 1) * 512)
                (eng or nc.sync).dma_start(
                    xt[:, :, sl],
                    xT[:, sl].rearrange("(t p) f -> p t f", p=128))
            nc.sync.dma_start(
                wqk[:, :, 0:256],
                wqkT[:, 0:256].rearrange("(t p) f -> p t f", p=128))
            load_x(0)
            nc.sync.dma_start(wv[:], wvT.rearrange("(t p) f -> p t f", p=128))
            load_x(1)
            load_x(2)
            load_x(3)
            nc.sync.dma_start(
                wqk[:, :, 256:384],
                wqkT[:, 256:384].rearrange("(t p) f -> p t f", p=128))
            nc.sync.dma_start(wp01[:], wpT[0:128, :])
            nc.sync.dma_start(wp2[:], wpT[128:192, :])

            qkv_ps = tc.alloc_tile_pool(name="qkv_ps", bufs=1, space="PSUM")
            st_ps = tc.alloc_tile_pool(name="st_ps", bufs=1, space="PSUM")
            av_ps = tc.alloc_tile_pool(name="av_ps", bufs=1, space="PSUM")
            apool = tc.alloc_tile_pool(name="a_sb", bufs=1)
            scr = tc.alloc_tile_pool(name="scr", bufs=1)

            # ---- filler closures (each emits one psum group of work) ----
            def QKg(ch, fi):
                def f(state):
                    sl = slice(ch * 512, (ch + 1) * 512)
                    dst = (qk0, qk1)[fi]
                    ps = qkv_ps.tile([128, 512], F32, tag="qkvps", bufs=2)
                    for ct in range(CT):
                        nc.tensor.matmul(
                            ps[:], wqk[:, ct, fi * 128:(fi + 1) * 128],
                            xt[:, ct, sl],
                            start=(ct == 0), stop=(ct == CT - 1))
                    nc.vector.tensor_copy(dst[:, sl], ps[:])
                return f

            def Vg(nt):
                def f(state):
                    ps = qkv_ps.tile([128, 512], F32, tag="qkvps", bufs=2)
                    for ct in range(CT):
                        nc.tensor.matmul(
                            ps[:, 0:NH * D], xt[:, ct, nt * 128:(nt + 1) * 128],
                            wv[:, ct, :],
                            start=(ct == 0), stop=(ct == CT - 1))
                    nc.vector.tensor_copy(
                        vsb[:, nt, :, :],
                        ps[:, 0:NH * D].rearrange("p (h x) -> p h x", x=64))
                return f

            def Q2g(ch):
                def f(state):
                    sl = slice(ch * 512, (ch + 1) * 512)
                    ps = qkv_ps.tile([128, 512], F32, tag="qkvps", bufs=2)
                    for ct in range(CT):
                        nc.tensor.matmul(
                            ps[:], wqk[:, ct, 256:384], xt[:, ct, sl],
                            start=(ct == 0), stop=(ct == CT - 1))
                    nc.vector.tensor_copy(qk2[:, sl], ps[:])
                return f

            def SHIFT(state):
                nc.sync.dma_start(qk2b[64:128, :], qk2[0:64, :])

            def TP(h, qt):
                """Transpose one token-major ao tile to feature-major.

                Uses a qkvps ring buffer: a transpose is a start=True matmul,
                which pending-zeroes its whole psum bank, so it must not share
                the bank where the L sums accumulate."""
                def f(state):
                    src = ao2 if h == 2 else ao01
                    reg = qkv_ps.tile([128, 512], F32, tag="qkvps", bufs=2)
                    nc.tensor.transpose(reg[:, 0:128], src[:, qt, :], ident[:])
                    if h == 2:
                        nc.vector.tensor_copy(
                            aoT2[:, qt * 128:(qt + 1) * 128], reg[0:64, 0:128])
                    else:
                        nc.vector.tensor_copy(
                            aoT01[:, qt * 128:(qt + 1) * 128], reg[:, 0:128])
                return f

            def PJg(co, ch, act_copy=False):
                def f(state):
                    csl = slice(co * 128, (co + 1) * 128)
                    sl = slice(ch * 512, (ch + 1) * 512)
                    ps = qkv_ps.tile([128, 512], F32, tag="qkvps", bufs=2)
                    nc.tensor.matmul(ps[:], wp01[:, csl], aoT01[:, sl],
                                     start=True, stop=False)
                    nc.tensor.matmul(ps[:], wp2[:, csl], aoT2[:, sl],
                                     start=False, stop=True)
                    if act_copy:
                        nc.scalar.activation(otile[co][:, sl], ps[:], CPY)
                    else:
                        nc.vector.tensor_copy(otile[co][:, sl], ps[:])
                return f

            def alloc_state():
                av = av_ps.tile([128, 8, 64], F32, tag="av", bufs=1)
                # lt: cols 0:8 L sums, 8:12 diag scratch (one psum bank).
                lt = av_ps.tile([128, 16], F32, tag="lt", bufs=1)
                return {"av": av, "lt": lt, "pq": [], "a2": None}

            def emit_diag(h, q_ap, k_ap, base, state):
                """Per-token diagonal q.k for the unseen range, exp'd to esb."""
                lt = state["lt"]
                pr = prod[h][base:base + 64, 0:u]
                nc.vector.tensor_tensor(
                    out=pr, in0=q_ap[:, kfull:N], in1=k_ap[:, kfull:N], op=MUL)
                # start=False: these bytes are still pending-zero from the L
                # bank's group start, so the first write stores cleanly, and
                # we must not wipe the accumulated L columns.
                for j in range(NU):
                    nc.tensor.matmul(
                        lt[:, 8 + j:9 + j],
                        prod[h][base:base + 64, j * 128:(j + 1) * 128],
                        ones1[base:base + 64, :], start=False, stop=False,
                        skip_group_check=True)
                nc.scalar.activation(esb[h][:, 0:NU], lt[:, 8:8 + NU],
                                     EXP, scale=0.125)

            AVLAG = 3

            def emit_av_block(h, state):
                t, a = state["pq"].pop(0)
                av, lt = state["av"], state["lt"]
                # start=True pending-zeroes the whole 2KB psum bank, so only
                # the first matmul touching each bank may set it; later
                # regions' first writes land on pending-zero bytes and store
                # (not accumulate) -- exactly what a fresh group needs.
                for qs in range(8):
                    asub = a[:, qs * 128:(qs + 1) * 128]
                    nc.tensor.matmul(
                        av[:, qs, :], asub, vsb[:, t, h, :],
                        start=(t == 0 and qs == 0),
                        stop=(t == T - 1 and qs == 7),
                        skip_group_check=True)
                    nc.tensor.matmul(
                        lt[:, qs:qs + 1], asub, ones1[:],
                        start=(t == 0 and qs == 0),
                        stop=(t == T - 1 and qs == 7),
                        skip_group_check=True)

            pending = {}

            def finish_pending(part=3, norm_split=False):
                """Flush the previous chunk's lagged AV blocks, its diag
                fix, and its normalization, spread over the next chunk's
                first slots: part=0/1 drain AV blocks, part=2 does diag+norm,
                part=3 does everything remaining."""
                if not pending:
                    return
                h, qc, q_ap, k_ap, base, state = pending["v"]
                if part == 0:
                    for _ in range(len(state["pq"]) // 2):
                        emit_av_block(h, state)
                    return
                if part == 1:
                    while state["pq"]:
                        emit_av_block(h, state)
                    return
                del pending["v"]
                while state["pq"]:
                    emit_av_block(h, state)
                if u and qc == 1:
                    emit_diag(h, q_ap, k_ap, base, state)
                emit_norm(h, qc, state, norm_split)

            def emit_attn_chunk(h, q_ap, k_ap, qc, t_lo, t_hi, state,
                                fillers=None):
                """ST+exp for key tiles [t_lo, t_hi); AV lags AVLAG steps.

                fillers: dict t -> list of closures drained after ST(t)."""
                for t in range(t_lo, t_hi):
                    st = st_ps.tile([128, 1024], F32, tag="st", bufs=2)
                    for cc in range(2):
                        qsl = slice(qc * 1024 + cc * 512,
                                    qc * 1024 + cc * 512 + 512)
                        nc.tensor.matmul(
                            st[:, cc * 512:(cc + 1) * 512],
                            k_ap[:, t * 128:t * 128 + 128],
                            q_ap[:, qsl], start=True, stop=True)
                    if t == t_lo:
                        finish_pending(0)
                    elif t == t_lo + 1:
                        finish_pending(3)
                    if fillers:
                        for fn in fillers.get(t, ()):
                            fn(state)
                    if len(state["pq"]) > AVLAG:
                        emit_av_block(h, state)
                    a = apool.tile([128, 1024], BF16, tag="a",
                                   bufs=AVLAG + 2)
                    nc.scalar.activation(a[:], st[:], EXP, scale=0.125)
                    state["pq"].append((t, a))

            def emit_norm(h, qc, state, norm_split=False):
                """Denominator fixup + batched reciprocal + normalization."""
                av, lt = state["av"], state["lt"]
                dsttile = ao2 if h == 2 else ao01
                dstoff = 64 if h == 1 else 0
                for qs in range(8):
                    qt = qc * 8 + qs
                    if qt >= ut0:
                        e = esb[h][:, qt - ut0:qt - ut0 + 1]
                        nc.vector.scalar_tensor_tensor(
                            out=av[:, qs, :], in0=vsb[:, qt, h, :],
                            scalar=e, in1=av[:, qs, :], op0=MUL, op1=ADD)
                u_lo = max(ut0 - qc * 8, 0)
                if u_lo < 8:  # add diag exp into the masked-tile L sums
                    nc.vector.tensor_tensor(
                        out=lt[:, u_lo:8], in0=lt[:, u_lo:8],
                        in1=esb[h][:, qc * 8 + u_lo - ut0:qc * 8 + 8 - ut0],
                        op=ADD)
                rec = scr.tile([128, 8], F32, tag="rec", bufs=2)
                nc.vector.reciprocal(rec[:], lt[:, 0:8])
                for qs in range(8):
                    qt = qc * 8 + qs
                    dst = dsttile[:, qt, dstoff:dstoff + 64]
                    if norm_split and qs % 2:
                        nc.scalar.activation(dst, av[:, qs, :], CPY,
                                             scale=rec[:, qs:qs + 1])
                    else:
                        nc.vector.tensor_scalar_mul(
                            dst, av[:, qs, :], rec[:, qs:qs + 1])

            # ---- filler schedules (tuned for T == 12; fallback: upfront) ----
            fill = {(h, qc): {} for h in range(NH) for qc in range(2)}
            flushf = {(h, qc): [] for h in range(NH) for qc in range(2)}

            def put(h, qc, t, fn):
                fill[(h, qc)].setdefault(t, []).append(fn)

            if T == 12:
                lead = [QKg(0, 0), QKg(1, 0), QKg(0, 1)]
                sched = {0: [Vg(0), Vg(1)], 1: [QKg(1, 1), Vg(2)],
                         2: [Vg(3)], 3: [Vg(4)], 4: [Vg(5)], 5: [Vg(6)],
                         6: [Vg(7)], 7: [QKg(2, 1), Vg(8)],
                         8: [QKg(2, 0), Vg(9)], 9: [QKg(3, 0), Vg(10)],
                         10: [Vg(11)]}
                for t, fns in sched.items():
                    for fn in fns:
                        put(0, 0, t, fn)
                put(0, 1, 0, QKg(3, 1))
                for i in range(4):
                    put(0, 1, i, Vg(12 + i))
                for ch in range(NCH):
                    put(1, 0, ch, Q2g(ch))
                put(1, 0, NCH, SHIFT)
                # transposes read the previous chunk's normalized ao, which
                # lands at slot 1 (finish_pending) -- schedule them from slot 2
                for i in range(8):
                    put(1, 1, 2 + i, TP(0, i))      # pair tiles of qc0
                for i in range(8):
                    put(2, 0, 2 + i, TP(0, 8 + i))  # pair tiles of qc1
                for i in range(8):
                    put(2, 1, 2 + i, TP(2, i))      # head-2 tiles of qc0
                for i in range(CT):
                    put(2, 1, 6 + i, PJg(i, 0))
                put(2, 1, 10, PJg(0, 1))
                put(2, 1, 11, PJg(1, 1))
                pj_tail = None
            else:
                lead = ([QKg(ch, fi) for ch in range(NCH) for fi in (0, 1)]
                        + [Vg(nt) for nt in range(QT)]
                        + [Q2g(ch) for ch in range(NCH)] + [SHIFT])
                pj_tail = [(co, ch, co % 2 == 1)
                           for ch in range(NCH) for co in range(CT)]

            # ---- emission ----
            for fn in lead:
                fn(None)

            heads = [(qk0[0:64, :], qk1[0:64, :], 0),
                     (qk0[64:128, :], qk1[64:128, :], 64),
                     (qk2b[64:128, :], qk2[64:128, :], 64)]

            for h, (q_ap, k_ap, base) in enumerate(heads):
                for qc in range(2):
                    state = alloc_state()
                    emit_attn_chunk(h, q_ap, k_ap, qc, 0, T, state,
                                    fill[(h, qc)])
                    pending["v"] = (h, qc, q_ap, k_ap, base, state)
                    if T != 12:
                        finish_pending()
                        # fallback: transpose finished tiles in place
                        if h == 1:
                            for i in range(8):
                                TP(0, qc * 8 + i)(state)
                        elif h == 2:
                            for i in range(8):
                                TP(2, qc * 8 + i)(state)
            finish_pending(norm_split=True)

            for _pool in (scr, apool, av_ps, st_ps, qkv_ps):
                _pool.release()

            # ---- tail: head-2 qc1 transposes + remaining projection ----
            with tc.tile_pool(name="tp_ps", bufs=1, space="PSUM") as tpps, \
                 tc.tile_pool(name="pj_ps", bufs=1, space="PSUM") as pjps, \
                 tc.tile_pool(name="ost", bufs=1) as ost:
                def tail_tp(qt):
                    reg = tpps.tile([128, 128], F32, tag="tp", bufs=2)
                    nc.tensor.transpose(reg[:], ao2[:, qt, :], ident[:])
                    if qt % 2:
                        nc.scalar.activation(
                            aoT2[:, qt * 128:(qt + 1) * 128], reg[0:64, :], CPY)
                    else:
                        nc.vector.tensor_copy(
                            aoT2[:, qt * 128:(qt + 1) * 128], reg[0:64, :])

                def tail_pj(co, ch, act):
                    csl = slice(co * 128, (co + 1) * 128)
                    sl = slice(ch * 512, (ch + 1) * 512)
                    ps = pjps.tile([128, 512], F32, tag="pj", bufs=3)
                    nc.tensor.matmul(ps[:], wp01[:, csl], aoT01[:, sl],
                                     start=True, stop=False)
                    nc.tensor.matmul(ps[:], wp2[:, csl], aoT2[:, sl],
                                     start=False, stop=True)
                    if act:
                        nc.scalar.activation(otile[co][:, sl], ps[:], CPY)
                    else:
                        nc.vector.tensor_copy(otile[co][:, sl], ps[:])

                if T == 12:
                    for co in range(2, CT):
                        tail_pj(co, 1, co % 2 == 0)
                    for qt in range(8, 16):
                        tail_tp(qt)
                    nalt = 0
                    qeng = [nc.sync, nc.scalar, nc.gpsimd]
                    for co in range(CT):
                        for ch in (2, 3):
                            tail_pj(co, ch, nalt % 2 == 0)
                            nalt += 1
                        csl = slice(co * 128, (co + 1) * 128)
                        qeng[co % 3].dma_start(yT[csl, :], otile[co][:])
                else:
                    for co, ch, act in pj_tail:
                        tail_pj(co, ch, act)
                    for co in range(CT):
                        csl = slice(co * 128, (co + 1) * 128)
                        nc.sync.dma_start(yT[csl, :], otile[co][:])

    nc.compile()
    return nc


def kernel(**inputs):
    global _last_results
    from concourse.bass_utils import run_bass_kernel_spmd

    x = np.asarray(inputs["x"], np.float32)
    w_qkv = np.asarray(inputs["w_qkv"], np.float32)
    w_proj = np.asarray(inputs["w_proj"], np.float32)
    b_proj = np.asarray(inputs["b_proj"], np.float32)
    u = int(np.asarray(inputs["unseen_size"]))
    B = x.shape[0]

    nc = _build(u)

    wT = np.ascontiguousarray(w_qkv.T).astype(NPBF)        # [768, 2304]
    wpT_full = np.ascontiguousarray(w_proj.T).astype(NPBF)  # [768(ci), 768(co)]
    xTb = [np.ascontiguousarray(x[b].T).astype(NPBF) for b in range(B)]

    in_maps = []
    for core in range(8):
        b, g = divmod(core, 4)
        hs = [3 * g, 3 * g + 1, 3 * g + 2]
        cols = []
        for h in hs[:2]:
            cols += [0 * C + h * D + i for i in range(D)]   # q0 q1
        for h in hs[:2]:
            cols += [1 * C + h * D + i for i in range(D)]   # k0 k1
        cols += [0 * C + hs[2] * D + i for i in range(D)]   # q2
        cols += [1 * C + hs[2] * D + i for i in range(D)]   # k2
        wqkT = np.ascontiguousarray(wT[:, cols])
        vcols = [2 * C + h * D + i for h in hs for i in range(D)]
        wvT = np.ascontiguousarray(wT[:, vcols])
        ci = [h * D + i for h in hs for i in range(D)]
        wpT = np.ascontiguousarray(wpT_full[ci, :])
        in_maps.append({"xT": xTb[b], "wqkT": wqkT, "wvT": wvT, "wpT": wpT})

    trace = bool(int(os.environ.get("KERNEL_TRACE", "0")))
    res = run_bass_kernel_spmd(nc, in_maps, core_ids=list(range(8)), trace=trace)
    _last_results = res

    y = np.zeros((B, N, C), np.float32)
    for core in range(8):
        b = core // 4
        y[b] += np.asarray(res.results[core]["yT"], np.float32).T
    y += b_proj
    return y


# revision 5
# speedup vs baseline: 1.0597x; 1.0597x over previous
"""MCCDecoderAttention Trainium2 kernel (8 NeuronCores), v3.

Sharding: core = b*4 + g  (b in {0,1} batch, g in {0..3} head-group).
Each core computes attention for 3 heads of one batch plus its partial
output projection; the host sums 4 partials per batch and adds b_proj.

Layout (all matmul operands bf16, f32 PSUM accumulation):
  - Q/K projected feature-major into packed 128-row tiles
    qk0=[q0;q1], qk1=[k0;k1], qk2=[q2;k2]; q2 is copied to partitions
    64:128 of qk2b by an SBUF->SBUF DMA so head 2's matmuls have
    matching base partitions.
  - V projected token-major straight into vsb[key, head, 0:64].
  - Scores S^T computed per (head, 1024-query chunk, 128-key tile);
    exp on ScalarE (scale=1/8 folded, no max subtraction needed).
  - AV uses the transposed form: lhsT = A^T subtile [128k x 128q],
    rhs = vsb [128k x 64] -> psum [128q, 64] accumulated over key
    tiles; the softmax denominator L accumulates in parallel via
    1-column ones matmuls.  The narrow free dims halve PE cost vs the
    feature-major AV form.
  - Decoder mask: keys limited to [0, N-u); each unseen query gets its
    diagonal term back via a 1-column ones-matmul of q*k (elementwise),
    exp, and a fused DVE multiply-add into the psum accumulator.
  - Normalized ao is written token-major f32 with heads 0/1 interleaved
    per query tile; PE transposes [128q x 128c] tiles (into spare
    columns of the L psum bank) flip it feature-major for the output
    projection, which contracts [128ci]+[64ci] per psum group.

Scheduling: emission is software-pipelined (ST(t) is emitted before the
AV block of t-1 so the PE never queues behind the exp dependency), and
all projection / transpose / phase-3 work is drained as per-key-tile
"fillers" inside the Act-bound attention loops.
"""

import functools
import os
import sys

for _p in ("/opt/trn_rl_repo", "/root/.axon_site/_ro/trn_rl_repo"):
    if os.path.isdir(_p) and _p not in sys.path:
        sys.path.insert(0, _p)

import numpy as np

import concourse.bacc as bacc
import concourse.tile as tile
from concourse import mybir

N, C, D = 2048, 768, 64
NH = 3            # heads per core
CT = C // 128     # 6 contraction tiles
F32 = mybir.dt.float32
BF16 = mybir.dt.bfloat16
FP8 = mybir.dt.float8e4
DR = mybir.MatmulPerfMode.DoubleRow
NPBF = mybir.dt.np(BF16)
EXP = mybir.ActivationFunctionType.Exp
CPY = mybir.ActivationFunctionType.Copy
MUL = mybir.AluOpType.mult
ADD = mybir.AluOpType.add

_last_results = None  # BassKernelResults of the most recent run (for test.py)


@functools.lru_cache(maxsize=4)
def _build(u: int):
    assert u % 128 == 0 and 0 <= u <= 512, f"unsupported unseen_size {u}"
    nc = bacc.Bacc(None, target_bir_lowering=False)
    xT = nc.dram_tensor("xT", [C, N], BF16, kind="ExternalInput")
    wqkT = nc.dram_tensor("wqkT", [C, 6 * D], BF16, kind="ExternalInput")
    wvT = nc.dram_tensor("wvT", [C, NH * D], BF16, kind="ExternalInput")
    wpT = nc.dram_tensor("wpT", [NH * D, C], BF16, kind="ExternalInput")
    yT = nc.dram_tensor("yT", [C, N], BF16, kind="ExternalOutput")

    kfull = N - u
    T = (kfull + 127) // 128           # key tiles
    QT = N // 128                      # 16 query tiles
    ut0 = kfull // 128                 # first unseen query tile
    NCH = N // 512                     # 4 token chunks
    NU = u // 128                      # unseen tiles

    with nc.allow_low_precision(reason="bf16 staging/outputs"), \
         tile.TileContext(nc) as tc:
        with tc.tile_pool(name="persist", bufs=1) as P:
            xt = P.tile([128, CT, N], BF16)
            wqk = P.tile([128, CT, 6 * D], BF16)
            wv = P.tile([128, CT, NH * D], BF16)
            wp01 = P.tile([128, C], BF16)
            wp2 = P.tile([64, C], BF16)
            qk0 = P.tile([128, N], BF16)   # [q0; q1]
            qk1 = P.tile([128, N], BF16)   # [k0; k1]
            qk2 = P.tile([128, N], BF16)   # [q2; k2]
            qk2b = P.tile([128, N], BF16)  # [-; q2] (dma partition shift)
            vsb = P.tile([128, QT, NH, 64], BF16)
            # token-major normalized attention out: heads 0/1 interleaved per
            # query tile; head 2 in the low half of its own tile (high half
            # junk, transposed but never copied out).
            ao01 = P.tile([128, QT, 128], F32)
            ao2 = P.tile([128, QT, 128], F32)
            aoT01 = P.tile([128, N], BF16)
            aoT2 = P.tile([64, N], BF16)
            otile = [P.tile([128, N], BF16, name=f"ot{co}", tag=f"ot{co}")
                     for co in range(CT)]
            ones1 = P.tile([128, 1], BF16)
            ident = P.tile([128, 128], F32)
            idones = P.tile([128, 128], F32)
            prod = [P.tile([128, max(u, 1)], BF16, name=f"pr{h}", tag=f"pr{h}")
                    for h in range(NH)] if u else []
            esb = [P.tile([128, max(NU, 1)], F32, name=f"e{h}", tag=f"e{h}")
                   for h in range(NH)] if u else []

            nc.vector.memset(ones1[:], 1.0)
            nc.vector.memset(idones[:], 1.0)
            nc.gpsimd.affine_select(
                ident[:], idones[:], pattern=[[-1, 128]],
                compare_op=mybir.AluOpType.is_equal, fill=0.0,
                base=0, channel_multiplier=1)

            # loads: SP queue carries the QK critical path (wqk cols 0:256
            # then x chunks 0,1,3); the idle Act queue takes wv + x chunk 2.
            def load_x(ch, eng=None):
                sl = slice(ch * 512, (ch + 1) * 512)
                (eng or nc.sync).dma_start(
                    xt[:, :, sl],
                    xT[:, sl].rearrange("(t p) f -> p t f", p=128))
            nc.sync.dma_start(
                wqk[:, :, 0:256],
                wqkT[:, 0:256].rearrange("(t p) f -> p t f", p=128))
            load_x(0)
            nc.sync.dma_start(wv[:], wvT.rearrange("(t p) f -> p t f", p=128))
            load_x(1)
            load_x(2)
            load_x(3)
            nc.sync.dma_start(
                wqk[:, :, 256:384],
                wqkT[:, 256:384].rearrange("(t p) f -> p t f", p=128))
            nc.sync.dma_start(wp01[:], wpT[0:128, :])
            nc.sync.dma_start(wp2[:], wpT[128:192, :])

            qkv_ps = tc.alloc_tile_pool(name="qkv_ps", bufs=1, space="PSUM")
            st_ps = tc.alloc_tile_pool(name="st_ps", bufs=1, space="PSUM")
            av_ps = tc.alloc_tile_pool(name="av_ps", bufs=1, space="PSUM")
            apool = tc.alloc_tile_pool(name="a_sb", bufs=1)
            scr = tc.alloc_tile_pool(name="scr", bufs=1)

            # ---- filler closures (each emits one psum group of work) ----
            def QKg(ch, fi):
                def f(state):
                    sl = slice(ch * 512, (ch + 1) * 512)
                    dst = (qk0, qk1)[fi]
                    ps = qkv_ps.tile([128, 512], F32, tag="qkvps", bufs=2)
                    for ct in range(CT):
                        nc.tensor.matmul(
                            ps[:], wqk[:, ct, fi * 128:(fi + 1) * 128],
                            xt[:, ct, sl],
                            start=(ct == 0), stop=(ct == CT - 1))
                    nc.vector.tensor_copy(dst[:, sl], ps[:])
                return f

            def Vg(nt):
                def f(state):
                    ps = qkv_ps.tile([128, 512], F32, tag="qkvps", bufs=2)
                    for ct in range(CT):
                        nc.tensor.matmul(
                            ps[:, 0:NH * D], xt[:, ct, nt * 128:(nt + 1) * 128],
                            wv[:, ct, :],
                            start=(ct == 0), stop=(ct == CT - 1))
                    nc.vector.tensor_copy(
                        vsb[:, nt, :, :],
                        ps[:, 0:NH * D].rearrange("p (h x) -> p h x", x=64))
                return f

            def Q2g(ch):
                def f(state):
                    sl = slice(ch * 512, (ch + 1) * 512)
                    ps = qkv_ps.tile([128, 512], F32, tag="qkvps", bufs=2)
                    for ct in range(CT):
                        nc.tensor.matmul(
                            ps[:], wqk[:, ct, 256:384], xt[:, ct, sl],
                            start=(ct == 0), stop=(ct == CT - 1))
                    nc.vector.tensor_copy(qk2[:, sl], ps[:])
                return f

            def SHIFT(state):
                nc.sync.dma_start(qk2b[64:128, :], qk2[0:64, :])

            def TP(h, qt):
                """Transpose one token-major ao tile to feature-major.

                Uses a qkvps ring buffer: a transpose is a start=True matmul,
                which pending-zeroes its whole psum bank, so it must not share
                the bank where the L sums accumulate."""
                def f(state):
                    src = ao2 if h == 2 else ao01
                    reg = qkv_ps.tile([128, 512], F32, tag="qkvps", bufs=2)
                    nc.tensor.transpose(reg[:, 0:128], src[:, qt, :], ident[:])
                    if h == 2:
                        nc.vector.tensor_copy(
                            aoT2[:, qt * 128:(qt + 1) * 128], reg[0:64, 0:128])
                    else:
                        nc.vector.tensor_copy(
                            aoT01[:, qt * 128:(qt + 1) * 128], reg[:, 0:128])
                return f

            def PJg(co, ch, act_copy=False):
                def f(state):
                    csl = slice(co * 128, (co + 1) * 128)
                    sl = slice(ch * 512, (ch + 1) * 512)
                    ps = qkv_ps.tile([128, 512], F32, tag="qkvps", bufs=2)
                    nc.tensor.matmul(ps[:], wp01[:, csl], aoT01[:, sl],
                                     start=True, stop=False)
                    nc.tensor.matmul(ps[:], wp2[:, csl], aoT2[:, sl],
                                     start=False, stop=True)
                    if act_copy:
                        nc.scalar.activation(otile[co][:, sl], ps[:], CPY)
                    else:
                        nc.vector.tensor_copy(otile[co][:, sl], ps[:])
                return f

            def alloc_state():
                av = av_ps.tile([128, 8, 64], F32, tag="av", bufs=1)
                # lt: cols 0:8 L sums, 8:12 diag scratch (one psum bank).
                lt = av_ps.tile([128, 16], F32, tag="lt", bufs=1)
                return {"av": av, "lt": lt, "pq": [], "a2": None}

            def emit_diag(h, q_ap, k_ap, base, state):
                """Per-token diagonal q.k for the unseen range, exp'd to esb."""
                lt = state["lt"]
                pr = prod[h][base:base + 64, 0:u]
                nc.vector.tensor_tensor(
                    out=pr, in0=q_ap[:, kfull:N], in1=k_ap[:, kfull:N], op=MUL)
                # start=False: these bytes are still pending-zero from the L
                # bank's group start, so the first write stores cleanly, and
                # we must not wipe the accumulated L columns.
                for j in range(NU):
                    nc.tensor.matmul(
                        lt[:, 8 + j:9 + j],
                        prod[h][base:base + 64, j * 128:(j + 1) * 128],
                        ones1[base:base + 64, :], start=False, stop=False,
                        skip_group_check=True)
                nc.scalar.activation(esb[h][:, 0:NU], lt[:, 8:8 + NU],
                                     EXP, scale=0.125)

            AVLAG = 6

            def emit_av_block(h, state):
                t, a = state["pq"].pop(0)
                av, lt = state["av"], state["lt"]
                # start=True pending-zeroes the whole 2KB psum bank, so only
                # the first matmul touching each bank may set it; later
                # regions' first writes land on pending-zero bytes and store
                # (not accumulate) -- exactly what a fresh group needs.
                for qs in range(8):
                    asub = a[:, qs * 128:(qs + 1) * 128]
                    nc.tensor.matmul(
                        av[:, qs, :], asub, vsb[:, t, h, :],
                        start=(t == 0 and qs == 0),
                        stop=(t == T - 1 and qs == 7),
                        skip_group_check=True)
                    nc.tensor.matmul(
                        lt[:, qs:qs + 1], asub, ones1[:],
                        start=(t == 0 and qs == 0),
                        stop=(t == T - 1 and qs == 7),
                        skip_group_check=True)

            pending = {}

            def finish_pending(part=3, norm_split=False):
                """Flush the previous chunk's lagged AV blocks, its diag
                fix, and its normalization, spread over the next chunk's
                first slots: part=0/1 drain AV blocks, part=2 does diag+norm,
                part=3 does everything remaining."""
                if not pending:
                    return
                h, qc, q_ap, k_ap, base, state = pending["v"]
                if part == 0:
                    for _ in range(len(state["pq"]) // 2):
                        emit_av_block(h, state)
                    return
                if part == 1:
                    while state["pq"]:
                        emit_av_block(h, state)
                    return
                del pending["v"]
                while state["pq"]:
                    emit_av_block(h, state)
                if u and qc == 1:
                    emit_diag(h, q_ap, k_ap, base, state)
                emit_norm(h, qc, state, norm_split)

            def emit_attn_chunk(h, q_ap, k_ap, qc, t_lo, t_hi, state,
                                fillers=None, lag=None):
                """ST+exp for key tiles [t_lo, t_hi); AV lags `lag` steps.

                fillers: dict t -> list of closures drained after ST(t)."""
                if lag is None:
                    lag = AVLAG
                for t in range(t_lo, t_hi):
                    st = st_ps.tile([128, 1024], F32, tag="st", bufs=2)
                    for cc in range(2):
                        qsl = slice(qc * 1024 + cc * 512,
                                    qc * 1024 + cc * 512 + 512)
                        nc.tensor.matmul(
                            st[:, cc * 512:(cc + 1) * 512],
                            k_ap[:, t * 128:t * 128 + 128],
                            q_ap[:, qsl], start=True, stop=True)
                    if t == t_lo:
                        finish_pending(0)
                    elif t == t_lo + 1:
                        finish_pending(3)
                    if fillers:
                        for fn in fillers.get(t, ()):
                            fn(state)
                    if len(state["pq"]) > lag:
                        emit_av_block(h, state)
                    a = apool.tile([128, 1024], BF16, tag="a",
                                   bufs=AVLAG + 2)
                    nc.scalar.activation(a[:], st[:], EXP, scale=0.125)
                    state["pq"].append((t, a))

            def emit_norm(h, qc, state, norm_split=False):
                """Denominator fixup + batched reciprocal + normalization."""
                av, lt = state["av"], state["lt"]
                dsttile = ao2 if h == 2 else ao01
                dstoff = 64 if h == 1 else 0
                for qs in range(8):
                    qt = qc * 8 + qs
                    if qt >= ut0:
                        e = esb[h][:, qt - ut0:qt - ut0 + 1]
                        nc.vector.scalar_tensor_tensor(
                            out=av[:, qs, :], in0=vsb[:, qt, h, :],
                            scalar=e, in1=av[:, qs, :], op0=MUL, op1=ADD)
                u_lo = max(ut0 - qc * 8, 0)
                if u_lo < 8:  # add diag exp into the masked-tile L sums
                    nc.vector.tensor_tensor(
                        out=lt[:, u_lo:8], in0=lt[:, u_lo:8],
                        in1=esb[h][:, qc * 8 + u_lo - ut0:qc * 8 + 8 - ut0],
                        op=ADD)
                rec = scr.tile([128, 8], F32, tag="rec", bufs=2)
                nc.vector.reciprocal(rec[:], lt[:, 0:8])
                for qs in range(8):
                    qt = qc * 8 + qs
                    dst = dsttile[:, qt, dstoff:dstoff + 64]
                    if norm_split and qs % 2:
                        nc.scalar.activation(dst, av[:, qs, :], CPY,
                                             scale=rec[:, qs:qs + 1])
                    else:
                        nc.vector.tensor_scalar_mul(
                            dst, av[:, qs, :], rec[:, qs:qs + 1])

            # ---- filler schedules (tuned for T == 12; fallback: upfront) ----
            fill = {(h, qc): {} for h in range(NH) for qc in range(2)}
            flushf = {(h, qc): [] for h in range(NH) for qc in range(2)}

            def put(h, qc, t, fn):
                fill[(h, qc)].setdefault(t, []).append(fn)

            if T == 12:
                lead = [QKg(0, 0), QKg(0, 1), QKg(1, 0)]
                sched = {0: [Vg(0), Vg(1)], 1: [QKg(1, 1), Vg(2)],
                         2: [Vg(3)], 3: [Vg(4)], 4: [Vg(5)], 5: [Vg(6)],
                         6: [Vg(7)], 7: [QKg(2, 1), Vg(8)],
                         8: [QKg(2, 0), Vg(9)], 9: [QKg(3, 0), Vg(10)],
                         10: [Vg(11)]}
                for t, fns in sched.items():
                    for fn in fns:
                        put(0, 0, t, fn)
                put(0, 1, 8, QKg(3, 1))
                for i in range(4):
                    put(0, 1, 2 + i, Vg(12 + i))
                for ch in range(NCH):
                    put(1, 0, 2 + ch, Q2g(ch))
                put(1, 0, 6, SHIFT)
                # transposes read the previous chunk's normalized ao, which
                # lands at slot 1 (finish_pending) -- schedule them from slot 2
                for i in range(8):
                    put(1, 1, 2 + i, TP(0, i))      # pair tiles of qc0
                for i in range(8):
                    put(2, 0, 2 + i, TP(0, 8 + i))  # pair tiles of qc1
                for i in range(8):
                    put(2, 1, 2 + i, TP(2, i))      # head-2 tiles of qc0
                for i in range(CT):
                    put(2, 1, 6 + i, PJg(i, 0))
                put(2, 1, 10, PJg(0, 1))
                put(2, 1, 11, PJg(1, 1))
                pj_tail = None
            else:
                lead = ([QKg(ch, fi) for ch in range(NCH) for fi in (0, 1)]
                        + [Vg(nt) for nt in range(QT)]
                        + [Q2g(ch) for ch in range(NCH)] + [SHIFT])
                pj_tail = [(co, ch, co % 2 == 1)
                           for ch in range(NCH) for co in range(CT)]

            # ---- emission ----
            # dummy matmuls ramp the PE p-state during the initial DMA wait
            # (full clock needs 3us of continuous PE busy); they overlap the
            # x/wqk transfers and abut the first real projection group.
            NDUM = int(os.environ.get("KERNEL_NDUM", "8"))
            if NDUM:
                dmy = qkv_ps.tile([128, 512], F32, tag="qkvps", bufs=2)
                for _ in range(NDUM):
                    nc.tensor.matmul(dmy[0:1, 0:128], idones[0:1, 0:1],
                                     idones[0:1, 0:128], start=True, stop=True)
            for fn in lead:
                fn(None)

            heads = [(qk0[0:64, :], qk1[0:64, :], 0),
                     (qk0[64:128, :], qk1[64:128, :], 64),
                     (qk2b[64:128, :], qk2[64:128, :], 64)]

            for h, (q_ap, k_ap, base) in enumerate(heads):
                for qc in range(2):
                    state = alloc_state()
                    emit_attn_chunk(h, q_ap, k_ap, qc, 0, T, state,
                                    fill[(h, qc)],
                                    lag=3 if (h, qc) == (2, 1) else None)
                    pending["v"] = (h, qc, q_ap, k_ap, base, state)
                    if T != 12:
                        finish_pending()
                        # fallback: transpose finished tiles in place
                        if h == 1:
                            for i in range(8):
                                TP(0, qc * 8 + i)(state)
                        elif h == 2:
                            for i in range(8):
                                TP(2, qc * 8 + i)(state)
            finish_pending(norm_split=True)

            for _pool in (scr, apool, av_ps, st_ps, qkv_ps):
                _pool.release()

            # ---- tail: head-2 qc1 transposes + remaining projection ----
            with tc.tile_pool(name="tp_ps", bufs=1, space="PSUM") as tpps, \
                 tc.tile_pool(name="pj_ps", bufs=1, space="PSUM") as pjps, \
                 tc.tile_pool(name="ost", bufs=1) as ost:
                def tail_tp(qt):
                    reg = tpps.tile([128, 128], F32, tag="tp", bufs=2)
                    nc.tensor.transpose(reg[:], ao2[:, qt, :], ident[:])
                    if qt % 2:
                        nc.scalar.activation(
                            aoT2[:, qt * 128:(qt + 1) * 128], reg[0:64, :], CPY)
                    else:
                        nc.vector.tensor_copy(
                            aoT2[:, qt * 128:(qt + 1) * 128], reg[0:64, :])

                def tail_pj(co, ch, act):
                    csl = slice(co * 128, (co + 1) * 128)
                    sl = slice(ch * 512, (ch + 1) * 512)
                    ps = pjps.tile([128, 512], F32, tag="pj", bufs=3)
                    nc.tensor.matmul(ps[:], wp01[:, csl], aoT01[:, sl],
                                     start=True, stop=False)
                    nc.tensor.matmul(ps[:], wp2[:, csl], aoT2[:, sl],
                                     start=False, stop=True)
                    if act:
                        nc.scalar.activation(otile[co][:, sl], ps[:], CPY)
                    else:
                        nc.vector.tensor_copy(otile[co][:, sl], ps[:])

                if T == 12:
                    for co in range(2, CT):
                        tail_pj(co, 1, co % 2 == 0)
                    for qt in range(8, 16):
                        tail_tp(qt)
                    nalt = 0
                    qeng = [nc.sync, nc.scalar, nc.gpsimd]
                    for co in range(CT):
                        for ch in (2, 3):
                            tail_pj(co, ch, nalt % 2 == 0)
                            nalt += 1
                        csl = slice(co * 128, (co + 1) * 128)
                        if co == CT - 1:
                            nc.sync.dma_start(yT[csl, 0:1024],
                                              otile[co][:, 0:1024])
                            nc.scalar.dma_start(yT[csl, 1024:2048],
                                                otile[co][:, 1024:2048])
                        else:
                            qeng[co % 3].dma_start(yT[csl, :], otile[co][:])
                else:
                    for co, ch, act in pj_tail:
                        tail_pj(co, ch, act)
                    for co in range(CT):
                        csl = slice(co * 128, (co + 1) * 128)
                        nc.sync.dma_start(yT[csl, :], otile[co][:])

    nc.compile()
    return nc


def kernel(**inputs):
    global _last_results
    from concourse.bass_utils import run_bass_kernel_spmd

    x = np.asarray(inputs["x"], np.float32)
    w_qkv = np.asarray(inputs["w_qkv"], np.float32)
    w_proj = np.asarray(inputs["w_proj"], np.float32)
    b_proj = np.asarray(inputs["b_proj"], np.float32)
    u = int(np.asarray(inputs["unseen_size"]))
    B = x.shape[0]

    nc = _build(u)

    wT = np.ascontiguousarray(w_qkv.T).astype(NPBF)        # [768, 2304]
    wpT_full = np.ascontiguousarray(w_proj.T).astype(NPBF)  # [768(ci), 768(co)]
    xTb = [np.ascontiguousarray(x[b].T).astype(NPBF) for b in range(B)]

    in_maps = []
    for core in range(8):
        b, g = divmod(core, 4)
        hs = [3 * g, 3 * g + 1, 3 * g + 2]
        cols = []
        for h in hs[:2]:
            cols += [0 * C + h * D + i for i in range(D)]   # q0 q1
        for h in hs[:2]:
            cols += [1 * C + h * D + i for i in range(D)]   # k0 k1
        cols += [0 * C + hs[2] * D + i for i in range(D)]   # q2
        cols += [1 * C + hs[2] * D + i for i in range(D)]   # k2
        wqkT = np.ascontiguousarray(wT[:, cols])
        vcols = [2 * C + h * D + i for h in hs for i in range(D)]
        wvT = np.ascontiguousarray(wT[:, vcols])
        ci = [h * D + i for h in hs for i in range(D)]
        wpT = np.ascontiguousarray(wpT_full[ci, :])
        in_maps.append({"xT": xTb[b], "wqkT": wqkT, "wvT": wvT, "wpT": wpT})

    trace = bool(int(os.environ.get("KERNEL_TRACE", "0")))
    res = run_bass_kernel_spmd(nc, in_maps, core_ids=list(range(8)), trace=trace)
    _last_results = res

    y = np.zeros((B, N, C), np.float32)
    for core in range(8):
        b = core // 4
        y[b] += np.asarray(res.results[core]["yT"], np.float32).T
    y += b_proj
    return y


# revision 6
# speedup vs baseline: 1.0652x; 1.0052x over previous
"""MCCDecoderAttention Trainium2 kernel (8 NeuronCores), v3.

Sharding: core = b*4 + g  (b in {0,1} batch, g in {0..3} head-group).
Each core computes attention for 3 heads of one batch plus its partial
output projection; the host sums 4 partials per batch and adds b_proj.

Layout (all matmul operands bf16, f32 PSUM accumulation):
  - Q/K projected feature-major into packed 128-row tiles
    qk0=[q0;q1], qk1=[k0;k1], qk2=[q2;k2]; q2 is copied to partitions
    64:128 of qk2b by an SBUF->SBUF DMA so head 2's matmuls have
    matching base partitions.
  - V projected token-major straight into vsb[key, head, 0:64].
  - Scores S^T computed per (head, 1024-query chunk, 128-key tile);
    exp on ScalarE (scale=1/8 folded, no max subtraction needed).
  - AV uses the transposed form: lhsT = A^T subtile [128k x 128q],
    rhs = vsb [128k x 64] -> psum [128q, 64] accumulated over key
    tiles; the softmax denominator L accumulates in parallel via
    1-column ones matmuls.  The narrow free dims halve PE cost vs the
    feature-major AV form.
  - Decoder mask: keys limited to [0, N-u); each unseen query gets its
    diagonal term back via a 1-column ones-matmul of q*k (elementwise),
    exp, and a fused DVE multiply-add into the psum accumulator.
  - Normalized ao is written token-major f32 with heads 0/1 interleaved
    per query tile; PE transposes [128q x 128c] tiles (into spare
    columns of the L psum bank) flip it feature-major for the output
    projection, which contracts [128ci]+[64ci] per psum group.

Scheduling: emission is software-pipelined (ST(t) is emitted before the
AV block of t-1 so the PE never queues behind the exp dependency), and
all projection / transpose / phase-3 work is drained as per-key-tile
"fillers" inside the Act-bound attention loops.
"""

import functools
import os
import sys

for _p in ("/opt/trn_rl_repo", "/root/.axon_site/_ro/trn_rl_repo"):
    if os.path.isdir(_p) and _p not in sys.path:
        sys.path.insert(0, _p)

import numpy as np

import concourse.bacc as bacc
import concourse.tile as tile
from concourse import mybir

N, C, D = 2048, 768, 64
NH = 3            # heads per core
CT = C // 128     # 6 contraction tiles
F32 = mybir.dt.float32
BF16 = mybir.dt.bfloat16
FP8 = mybir.dt.float8e4
DR = mybir.MatmulPerfMode.DoubleRow
NPBF = mybir.dt.np(BF16)
EXP = mybir.ActivationFunctionType.Exp
CPY = mybir.ActivationFunctionType.Copy
MUL = mybir.AluOpType.mult
ADD = mybir.AluOpType.add

_last_results = None  # BassKernelResults of the most recent run (for test.py)


@functools.lru_cache(maxsize=4)
def _build(u: int):
    assert u % 128 == 0 and 0 <= u <= 512, f"unsupported unseen_size {u}"
    nc = bacc.Bacc(None, target_bir_lowering=False)
    xT = nc.dram_tensor("xT", [C, N], BF16, kind="ExternalInput")
    wqkT = nc.dram_tensor("wqkT", [C, 6 * D], BF16, kind="ExternalInput")
    wvT = nc.dram_tensor("wvT", [C, NH * D], BF16, kind="ExternalInput")
    wpT = nc.dram_tensor("wpT", [NH * D, C], BF16, kind="ExternalInput")
    yT = nc.dram_tensor("yT", [C, N], BF16, kind="ExternalOutput")

    kfull = N - u
    T = (kfull + 127) // 128           # key tiles
    QT = N // 128                      # 16 query tiles
    ut0 = kfull // 128                 # first unseen query tile
    NCH = N // 512                     # 4 token chunks
    NU = u // 128                      # unseen tiles

    with nc.allow_low_precision(reason="bf16 staging/outputs"), \
         tile.TileContext(nc) as tc:
        with tc.tile_pool(name="persist", bufs=1) as P:
            xt = P.tile([128, CT, N], BF16)
            wqk = P.tile([128, CT, 6 * D], BF16)
            wv = P.tile([128, CT, NH * D], BF16)
            wp01 = P.tile([128, C], BF16)
            wp2 = P.tile([64, C], BF16)
            qk0 = P.tile([128, N], BF16)   # [q0; q1]
            qk1 = P.tile([128, N], BF16)   # [k0; k1]
            qk2 = P.tile([128, N], BF16)   # [q2; k2]
            qk2b = P.tile([128, N], BF16)  # [-; q2] (dma partition shift)
            vsb = P.tile([128, QT, NH, 64], BF16)
            # token-major normalized attention out: heads 0/1 interleaved per
            # query tile; head 2 in the low half of its own tile (high half
            # junk, transposed but never copied out).
            ao01 = P.tile([128, QT, 128], F32)
            ao2 = P.tile([128, QT, 128], F32)
            aoT01 = P.tile([128, N], BF16)
            aoT2 = P.tile([64, N], BF16)
            otile = [P.tile([128, N], BF16, name=f"ot{co}", tag=f"ot{co}")
                     for co in range(CT)]
            ones1 = P.tile([128, 1], BF16)
            ident = P.tile([128, 128], F32)
            idones = P.tile([128, 128], F32)
            prod = [P.tile([128, max(u, 1)], BF16, name=f"pr{h}", tag=f"pr{h}")
                    for h in range(NH)] if u else []
            esb = [P.tile([128, max(NU, 1)], F32, name=f"e{h}", tag=f"e{h}")
                   for h in range(NH)] if u else []

            nc.vector.memset(ones1[:], 1.0)
            nc.vector.memset(idones[:], 1.0)
            nc.gpsimd.affine_select(
                ident[:], idones[:], pattern=[[-1, 128]],
                compare_op=mybir.AluOpType.is_equal, fill=0.0,
                base=0, channel_multiplier=1)

            # loads: SP queue carries the QK critical path (wqk cols 0:256
            # then x chunks 0,1,3); the idle Act queue takes wv + x chunk 2.
            def load_x(ch, eng=None):
                sl = slice(ch * 512, (ch + 1) * 512)
                (eng or nc.sync).dma_start(
                    xt[:, :, sl],
                    xT[:, sl].rearrange("(t p) f -> p t f", p=128))
            nc.sync.dma_start(
                wqk[:, :, 0:256],
                wqkT[:, 0:256].rearrange("(t p) f -> p t f", p=128))
            nc.sync.dma_start(
                xt[:, :, 0:256],
                xT[:, 0:256].rearrange("(t p) f -> p t f", p=128))
            nc.sync.dma_start(
                xt[:, :, 256:512],
                xT[:, 256:512].rearrange("(t p) f -> p t f", p=128))
            nc.sync.dma_start(wv[:], wvT.rearrange("(t p) f -> p t f", p=128))
            load_x(1)
            load_x(2)
            load_x(3)
            nc.sync.dma_start(
                wqk[:, :, 256:384],
                wqkT[:, 256:384].rearrange("(t p) f -> p t f", p=128))
            nc.sync.dma_start(wp01[:], wpT[0:128, :])
            nc.sync.dma_start(wp2[:], wpT[128:192, :])

            qkv_ps = tc.alloc_tile_pool(name="qkv_ps", bufs=1, space="PSUM")
            st_ps = tc.alloc_tile_pool(name="st_ps", bufs=1, space="PSUM")
            av_ps = tc.alloc_tile_pool(name="av_ps", bufs=1, space="PSUM")
            apool = tc.alloc_tile_pool(name="a_sb", bufs=1)
            scr = tc.alloc_tile_pool(name="scr", bufs=1)

            # ---- filler closures (each emits one psum group of work) ----
            def QKg(ch, fi, half=None):
                def f(state):
                    if half is None:
                        sl = slice(ch * 512, (ch + 1) * 512)
                    else:
                        sl = slice(ch * 512 + half * 256,
                                   ch * 512 + half * 256 + 256)
                    w = 512 if half is None else 256
                    dst = (qk0, qk1)[fi]
                    ps = qkv_ps.tile([128, 512], F32, tag="qkvps", bufs=2)
                    for ct in range(CT):
                        nc.tensor.matmul(
                            ps[:, 0:w], wqk[:, ct, fi * 128:(fi + 1) * 128],
                            xt[:, ct, sl],
                            start=(ct == 0), stop=(ct == CT - 1))
                    nc.vector.tensor_copy(dst[:, sl], ps[:, 0:w])
                return f

            def Vg(nt):
                def f(state):
                    ps = qkv_ps.tile([128, 512], F32, tag="qkvps", bufs=2)
                    for ct in range(CT):
                        nc.tensor.matmul(
                            ps[:, 0:NH * D], xt[:, ct, nt * 128:(nt + 1) * 128],
                            wv[:, ct, :],
                            start=(ct == 0), stop=(ct == CT - 1))
                    nc.vector.tensor_copy(
                        vsb[:, nt, :, :],
                        ps[:, 0:NH * D].rearrange("p (h x) -> p h x", x=64))
                return f

            def Q2g(ch):
                def f(state):
                    sl = slice(ch * 512, (ch + 1) * 512)
                    ps = qkv_ps.tile([128, 512], F32, tag="qkvps", bufs=2)
                    for ct in range(CT):
                        nc.tensor.matmul(
                            ps[:], wqk[:, ct, 256:384], xt[:, ct, sl],
                            start=(ct == 0), stop=(ct == CT - 1))
                    nc.vector.tensor_copy(qk2[:, sl], ps[:])
                return f

            def SHIFT(state):
                nc.sync.dma_start(qk2b[64:128, :], qk2[0:64, :])

            def TP(h, qt):
                """Transpose one token-major ao tile to feature-major.

                Uses a qkvps ring buffer: a transpose is a start=True matmul,
                which pending-zeroes its whole psum bank, so it must not share
                the bank where the L sums accumulate."""
                def f(state):
                    src = ao2 if h == 2 else ao01
                    reg = qkv_ps.tile([128, 512], F32, tag="qkvps", bufs=2)
                    nc.tensor.transpose(reg[:, 0:128], src[:, qt, :], ident[:])
                    if h == 2:
                        nc.vector.tensor_copy(
                            aoT2[:, qt * 128:(qt + 1) * 128], reg[0:64, 0:128])
                    else:
                        nc.vector.tensor_copy(
                            aoT01[:, qt * 128:(qt + 1) * 128], reg[:, 0:128])
                return f

            def PJg(co, ch, act_copy=False):
                def f(state):
                    csl = slice(co * 128, (co + 1) * 128)
                    sl = slice(ch * 512, (ch + 1) * 512)
                    ps = qkv_ps.tile([128, 512], F32, tag="qkvps", bufs=2)
                    nc.tensor.matmul(ps[:], wp01[:, csl], aoT01[:, sl],
                                     start=True, stop=False)
                    nc.tensor.matmul(ps[:], wp2[:, csl], aoT2[:, sl],
                                     start=False, stop=True)
                    if act_copy:
                        nc.scalar.activation(otile[co][:, sl], ps[:], CPY)
                    else:
                        nc.vector.tensor_copy(otile[co][:, sl], ps[:])
                return f

            def alloc_state():
                av = av_ps.tile([128, 8, 64], F32, tag="av", bufs=1)
                # lt: cols 0:8 L sums, 8:12 diag scratch (one psum bank).
                lt = av_ps.tile([128, 16], F32, tag="lt", bufs=1)
                return {"av": av, "lt": lt, "pq": [], "a2": None}

            def emit_diag(h, q_ap, k_ap, base, state):
                """Per-token diagonal q.k for the unseen range, exp'd to esb."""
                lt = state["lt"]
                pr = prod[h][base:base + 64, 0:u]
                nc.vector.tensor_tensor(
                    out=pr, in0=q_ap[:, kfull:N], in1=k_ap[:, kfull:N], op=MUL)
                # start=False: these bytes are still pending-zero from the L
                # bank's group start, so the first write stores cleanly, and
                # we must not wipe the accumulated L columns.
                for j in range(NU):
                    nc.tensor.matmul(
                        lt[:, 8 + j:9 + j],
                        prod[h][base:base + 64, j * 128:(j + 1) * 128],
                        ones1[base:base + 64, :], start=False, stop=False,
                        skip_group_check=True)
                nc.scalar.activation(esb[h][:, 0:NU], lt[:, 8:8 + NU],
                                     EXP, scale=0.125)

            AVLAG = 6

            def emit_av_block(h, state):
                t, a = state["pq"].pop(0)
                av, lt = state["av"], state["lt"]
                # start=True pending-zeroes the whole 2KB psum bank, so only
                # the first matmul touching each bank may set it; later
                # regions' first writes land on pending-zero bytes and store
                # (not accumulate) -- exactly what a fresh group needs.
                for qs in range(8):
                    asub = a[:, qs * 128:(qs + 1) * 128]
                    nc.tensor.matmul(
                        av[:, qs, :], asub, vsb[:, t, h, :],
                        start=(t == 0 and qs == 0),
                        stop=(t == T - 1 and qs == 7),
                        skip_group_check=True)
                    nc.tensor.matmul(
                        lt[:, qs:qs + 1], asub, ones1[:],
                        start=(t == 0 and qs == 0),
                        stop=(t == T - 1 and qs == 7),
                        skip_group_check=True)

            pending = {}

            def finish_pending(part=3, norm_split=False):
                """Flush the previous chunk's lagged AV blocks, its diag
                fix, and its normalization, spread over the next chunk's
                first slots: part=0/1 drain AV blocks, part=2 does diag+norm,
                part=3 does everything remaining."""
                if not pending:
                    return
                h, qc, q_ap, k_ap, base, state = pending["v"]
                if part == 0:
                    for _ in range(len(state["pq"]) // 2):
                        emit_av_block(h, state)
                    return
                if part == 1:
                    while state["pq"]:
                        emit_av_block(h, state)
                    return
                del pending["v"]
                while state["pq"]:
                    emit_av_block(h, state)
                if u and qc == 1:
                    emit_diag(h, q_ap, k_ap, base, state)
                emit_norm(h, qc, state, norm_split)

            def emit_attn_chunk(h, q_ap, k_ap, qc, t_lo, t_hi, state,
                                fillers=None, lag=None):
                """ST+exp for key tiles [t_lo, t_hi); AV lags `lag` steps.

                fillers: dict t -> list of closures drained after ST(t)."""
                if lag is None:
                    lag = AVLAG
                for t in range(t_lo, t_hi):
                    st = st_ps.tile([128, 1024], F32, tag="st", bufs=2)
                    for cc in range(2):
                        qsl = slice(qc * 1024 + cc * 512,
                                    qc * 1024 + cc * 512 + 512)
                        nc.tensor.matmul(
                            st[:, cc * 512:(cc + 1) * 512],
                            k_ap[:, t * 128:t * 128 + 128],
                            q_ap[:, qsl], start=True, stop=True)
                    if t == t_lo:
                        finish_pending(0)
                    elif t == t_lo + 1:
                        finish_pending(3)
                    if fillers:
                        for fn in fillers.get(t, ()):
                            fn(state)
                    if len(state["pq"]) > lag:
                        emit_av_block(h, state)
                    a = apool.tile([128, 1024], BF16, tag="a",
                                   bufs=AVLAG + 2)
                    nc.scalar.activation(a[:], st[:], EXP, scale=0.125)
                    state["pq"].append((t, a))

            def emit_norm(h, qc, state, norm_split=False):
                """Denominator fixup + batched reciprocal + normalization."""
                av, lt = state["av"], state["lt"]
                dsttile = ao2 if h == 2 else ao01
                dstoff = 64 if h == 1 else 0
                for qs in range(8):
                    qt = qc * 8 + qs
                    if qt >= ut0:
                        e = esb[h][:, qt - ut0:qt - ut0 + 1]
                        nc.vector.scalar_tensor_tensor(
                            out=av[:, qs, :], in0=vsb[:, qt, h, :],
                            scalar=e, in1=av[:, qs, :], op0=MUL, op1=ADD)
                u_lo = max(ut0 - qc * 8, 0)
                if u_lo < 8:  # add diag exp into the masked-tile L sums
                    nc.vector.tensor_tensor(
                        out=lt[:, u_lo:8], in0=lt[:, u_lo:8],
                        in1=esb[h][:, qc * 8 + u_lo - ut0:qc * 8 + 8 - ut0],
                        op=ADD)
                rec = scr.tile([128, 8], F32, tag="rec", bufs=2)
                nc.vector.reciprocal(rec[:], lt[:, 0:8])
                for qs in range(8):
                    qt = qc * 8 + qs
                    dst = dsttile[:, qt, dstoff:dstoff + 64]
                    if norm_split and qs % 2:
                        nc.scalar.activation(dst, av[:, qs, :], CPY,
                                             scale=rec[:, qs:qs + 1])
                    else:
                        nc.vector.tensor_scalar_mul(
                            dst, av[:, qs, :], rec[:, qs:qs + 1])

            # ---- filler schedules (tuned for T == 12; fallback: upfront) ----
            fill = {(h, qc): {} for h in range(NH) for qc in range(2)}
            flushf = {(h, qc): [] for h in range(NH) for qc in range(2)}

            def put(h, qc, t, fn):
                fill[(h, qc)].setdefault(t, []).append(fn)

            if T == 12:
                lead = [QKg(0, 0, 0), QKg(0, 0, 1), QKg(0, 1, 0),
                        QKg(0, 1, 1), QKg(1, 0, 0), QKg(1, 0, 1)]
                sched = {0: [Vg(0), Vg(1)], 1: [QKg(1, 1), Vg(2)],
                         2: [Vg(3)], 3: [Vg(4)], 4: [Vg(5)], 5: [Vg(6)],
                         6: [Vg(7)], 7: [QKg(2, 1), Vg(8)],
                         8: [QKg(2, 0), Vg(9)], 9: [QKg(3, 0), Vg(10)],
                         10: [Vg(11)]}
                for t, fns in sched.items():
                    for fn in fns:
                        put(0, 0, t, fn)
                put(0, 1, 8, QKg(3, 1))
                for i in range(4):
                    put(0, 1, 2 + i, Vg(12 + i))
                for ch in range(NCH):
                    put(1, 0, 2 + ch, Q2g(ch))
                put(1, 0, 6, SHIFT)
                # transposes read the previous chunk's normalized ao, which
                # lands at slot 1 (finish_pending) -- schedule them from slot 2
                for i in range(8):
                    put(1, 1, 2 + i, TP(0, i))      # pair tiles of qc0
                for i in range(8):
                    put(2, 0, 2 + i, TP(0, 8 + i))  # pair tiles of qc1
                for i in range(8):
                    put(2, 1, 2 + i, TP(2, i))      # head-2 tiles of qc0
                for i in range(CT):
                    put(2, 1, 6 + i, PJg(i, 0))
                put(2, 1, 10, PJg(0, 1))
                put(2, 1, 11, PJg(1, 1))
                pj_tail = None
            else:
                lead = ([QKg(ch, fi) for ch in range(NCH) for fi in (0, 1)]
                        + [Vg(nt) for nt in range(QT)]
                        + [Q2g(ch) for ch in range(NCH)] + [SHIFT])
                pj_tail = [(co, ch, co % 2 == 1)
                           for ch in range(NCH) for co in range(CT)]

            # ---- emission ----
            # dummy matmuls ramp the PE p-state during the initial DMA wait
            # (full clock needs 3us of continuous PE busy); they overlap the
            # x/wqk transfers and abut the first real projection group.
            NDUM = int(os.environ.get("KERNEL_NDUM", "8"))
            if NDUM:
                dmy = qkv_ps.tile([128, 512], F32, tag="qkvps", bufs=2)
                for _ in range(NDUM):
                    nc.tensor.matmul(dmy[0:1, 0:128], idones[0:1, 0:1],
                                     idones[0:1, 0:128], start=True, stop=True)
            for fn in lead:
                fn(None)

            heads = [(qk0[0:64, :], qk1[0:64, :], 0),
                     (qk0[64:128, :], qk1[64:128, :], 64),
                     (qk2b[64:128, :], qk2[64:128, :], 64)]

            for h, (q_ap, k_ap, base) in enumerate(heads):
                for qc in range(2):
                    state = alloc_state()
                    emit_attn_chunk(h, q_ap, k_ap, qc, 0, T, state,
                                    fill[(h, qc)],
                                    lag=3 if (h, qc) == (2, 1) else None)
                    pending["v"] = (h, qc, q_ap, k_ap, base, state)
                    if T != 12:
                        finish_pending()
                        # fallback: transpose finished tiles in place
                        if h == 1:
                            for i in range(8):
                                TP(0, qc * 8 + i)(state)
                        elif h == 2:
                            for i in range(8):
                                TP(2, qc * 8 + i)(state)
            finish_pending(norm_split=True)

            for _pool in (scr, apool, av_ps, st_ps, qkv_ps):
                _pool.release()

            # ---- tail: head-2 qc1 transposes + remaining projection ----
            with tc.tile_pool(name="tp_ps", bufs=1, space="PSUM") as tpps, \
                 tc.tile_pool(name="pj_ps", bufs=1, space="PSUM") as pjps, \
                 tc.tile_pool(name="ost", bufs=1) as ost:
                def tail_tp(qt):
                    reg = tpps.tile([128, 128], F32, tag="tp", bufs=2)
                    nc.tensor.transpose(reg[:], ao2[:, qt, :], ident[:])
                    if qt % 2:
                        nc.scalar.activation(
                            aoT2[:, qt * 128:(qt + 1) * 128], reg[0:64, :], CPY)
                    else:
                        nc.vector.tensor_copy(
                            aoT2[:, qt * 128:(qt + 1) * 128], reg[0:64, :])

                def tail_pj(co, ch, act):
                    csl = slice(co * 128, (co + 1) * 128)
                    sl = slice(ch * 512, (ch + 1) * 512)
                    ps = pjps.tile([128, 512], F32, tag="pj", bufs=3)
                    nc.tensor.matmul(ps[:], wp01[:, csl], aoT01[:, sl],
                                     start=True, stop=False)
                    nc.tensor.matmul(ps[:], wp2[:, csl], aoT2[:, sl],
                                     start=False, stop=True)
                    if act:
                        nc.scalar.activation(otile[co][:, sl], ps[:], CPY)
                    else:
                        nc.vector.tensor_copy(otile[co][:, sl], ps[:])

                if T == 12:
                    for co in range(2, CT):
                        tail_pj(co, 1, co % 2 == 0)
                    for qt in range(8, 16):
                        tail_tp(qt)
                    nalt = 0
                    qeng = [nc.sync, nc.scalar, nc.gpsimd]
                    for co in range(CT):
                        for ch in (2, 3):
                            tail_pj(co, ch, nalt % 2 == 0)
                            nalt += 1
                        csl = slice(co * 128, (co + 1) * 128)
                        if co == CT - 1:
                            nc.sync.dma_start(yT[csl, 0:1024],
                                              otile[co][:, 0:1024])
                            nc.scalar.dma_start(yT[csl, 1024:2048],
                                                otile[co][:, 1024:2048])
                        else:
                            qeng[co % 3].dma_start(yT[csl, :], otile[co][:])
                else:
                    for co, ch, act in pj_tail:
                        tail_pj(co, ch, act)
                    for co in range(CT):
                        csl = slice(co * 128, (co + 1) * 128)
                        nc.sync.dma_start(yT[csl, :], otile[co][:])

    nc.compile()
    return nc


def kernel(**inputs):
    global _last_results
    from concourse.bass_utils import run_bass_kernel_spmd

    x = np.asarray(inputs["x"], np.float32)
    w_qkv = np.asarray(inputs["w_qkv"], np.float32)
    w_proj = np.asarray(inputs["w_proj"], np.float32)
    b_proj = np.asarray(inputs["b_proj"], np.float32)
    u = int(np.asarray(inputs["unseen_size"]))
    B = x.shape[0]

    nc = _build(u)

    wT = np.ascontiguousarray(w_qkv.T).astype(NPBF)        # [768, 2304]
    wpT_full = np.ascontiguousarray(w_proj.T).astype(NPBF)  # [768(ci), 768(co)]
    xTb = [np.ascontiguousarray(x[b].T).astype(NPBF) for b in range(B)]

    in_maps = []
    for core in range(8):
        b, g = divmod(core, 4)
        hs = [3 * g, 3 * g + 1, 3 * g + 2]
        cols = []
        for h in hs[:2]:
            cols += [0 * C + h * D + i for i in range(D)]   # q0 q1
        for h in hs[:2]:
            cols += [1 * C + h * D + i for i in range(D)]   # k0 k1
        cols += [0 * C + hs[2] * D + i for i in range(D)]   # q2
        cols += [1 * C + hs[2] * D + i for i in range(D)]   # k2
        wqkT = np.ascontiguousarray(wT[:, cols])
        vcols = [2 * C + h * D + i for h in hs for i in range(D)]
        wvT = np.ascontiguousarray(wT[:, vcols])
        ci = [h * D + i for h in hs for i in range(D)]
        wpT = np.ascontiguousarray(wpT_full[ci, :])
        in_maps.append({"xT": xTb[b], "wqkT": wqkT, "wvT": wvT, "wpT": wpT})

    trace = bool(int(os.environ.get("KERNEL_TRACE", "0")))
    res = run_bass_kernel_spmd(nc, in_maps, core_ids=list(range(8)), trace=trace)
    _last_results = res

    y = np.zeros((B, N, C), np.float32)
    for core in range(8):
        b = core // 4
        y[b] += np.asarray(res.results[core]["yT"], np.float32).T
    y += b_proj
    return y


# revision 7
# speedup vs baseline: 1.0852x; 1.0188x over previous
"""MCCDecoderAttention Trainium2 kernel (8 NeuronCores), v3.

Sharding: core = b*4 + g  (b in {0,1} batch, g in {0..3} head-group).
Each core computes attention for 3 heads of one batch plus its partial
output projection; the host sums 4 partials per batch and adds b_proj.

Layout (all matmul operands bf16, f32 PSUM accumulation):
  - Q/K projected feature-major into packed 128-row tiles
    qk0=[q0;q1], qk1=[k0;k1], qk2=[q2;k2]; q2 is copied to partitions
    64:128 of qk2b by an SBUF->SBUF DMA so head 2's matmuls have
    matching base partitions.
  - V projected token-major straight into vsb[key, head, 0:64].
  - Scores S^T computed per (head, 1024-query chunk, 128-key tile);
    exp on ScalarE (scale=1/8 folded, no max subtraction needed).
  - AV uses the transposed form: lhsT = A^T subtile [128k x 128q],
    rhs = vsb [128k x 64] -> psum [128q, 64] accumulated over key
    tiles; the softmax denominator L accumulates in parallel via
    1-column ones matmuls.  The narrow free dims halve PE cost vs the
    feature-major AV form.
  - Decoder mask: keys limited to [0, N-u); each unseen query gets its
    diagonal term back via a 1-column ones-matmul of q*k (elementwise),
    exp, and a fused DVE multiply-add into the psum accumulator.
  - Normalized ao is written token-major f32 with heads 0/1 interleaved
    per query tile; PE transposes [128q x 128c] tiles (into spare
    columns of the L psum bank) flip it feature-major for the output
    projection, which contracts [128ci]+[64ci] per psum group.

Scheduling: emission is software-pipelined (ST(t) is emitted before the
AV block of t-1 so the PE never queues behind the exp dependency), and
all projection / transpose / phase-3 work is drained as per-key-tile
"fillers" inside the Act-bound attention loops.
"""

import functools
import os
import sys

for _p in ("/opt/trn_rl_repo", "/root/.axon_site/_ro/trn_rl_repo"):
    if os.path.isdir(_p) and _p not in sys.path:
        sys.path.insert(0, _p)

import numpy as np

import concourse.bacc as bacc
import concourse.tile as tile
from concourse import mybir

N, C, D = 2048, 768, 64
NH = 3            # heads per core
CT = C // 128     # 6 contraction tiles
F32 = mybir.dt.float32
BF16 = mybir.dt.bfloat16
FP8 = mybir.dt.float8e4
DR = mybir.MatmulPerfMode.DoubleRow
NPBF = mybir.dt.np(BF16)
EXP = mybir.ActivationFunctionType.Exp
CPY = mybir.ActivationFunctionType.Copy
MUL = mybir.AluOpType.mult
ADD = mybir.AluOpType.add

_last_results = None  # BassKernelResults of the most recent run (for test.py)


@functools.lru_cache(maxsize=4)
def _build(u: int):
    assert u % 128 == 0 and 0 <= u <= 512, f"unsupported unseen_size {u}"
    nc = bacc.Bacc(None, target_bir_lowering=False)
    xT = nc.dram_tensor("xT", [C, N], BF16, kind="ExternalInput")
    wqkT = nc.dram_tensor("wqkT", [C, 6 * D], BF16, kind="ExternalInput")
    wvT = nc.dram_tensor("wvT", [C, NH * D], BF16, kind="ExternalInput")
    wpT = nc.dram_tensor("wpT", [NH * D, C], BF16, kind="ExternalInput")
    yT = nc.dram_tensor("yT", [C, N], BF16, kind="ExternalOutput")

    kfull = N - u
    T = (kfull + 127) // 128           # key tiles
    QT = N // 128                      # 16 query tiles
    ut0 = kfull // 128                 # first unseen query tile
    NCH = N // 512                     # 4 token chunks
    NU = u // 128                      # unseen tiles

    with nc.allow_low_precision(reason="bf16 staging/outputs"), \
         tile.TileContext(nc) as tc:
        with tc.tile_pool(name="persist", bufs=1) as P:
            xt = P.tile([128, CT, N], BF16)
            wqk = P.tile([128, CT, 6 * D], BF16)
            wv = P.tile([128, CT, NH * D], BF16)
            wp01 = P.tile([128, C], BF16)
            wp2 = P.tile([64, C], BF16)
            qk0 = P.tile([128, N], BF16)   # [q0; q1]
            qk1 = P.tile([128, N], BF16)   # [k0; k1]
            qk2 = P.tile([128, N], BF16)   # [q2; k2]
            qk2b = P.tile([128, N], BF16)  # [-; q2] (dma partition shift)
            vsb = P.tile([128, QT, NH, 64], BF16)
            # token-major normalized attention out: heads 0/1 interleaved per
            # query tile; head 2 in the low half of its own tile (high half
            # junk, transposed but never copied out).
            ao01 = P.tile([128, QT, 128], F32)
            ao2 = P.tile([128, QT, 128], F32)
            aoT01 = P.tile([128, N], BF16)
            aoT2 = P.tile([64, N], BF16)
            otile = [P.tile([128, N], BF16, name=f"ot{co}", tag=f"ot{co}")
                     for co in range(CT)]
            ones1 = P.tile([128, 1], BF16)
            ident = P.tile([128, 128], F32)
            idones = P.tile([128, 128], F32)
            prod = [P.tile([128, max(u, 1)], BF16, name=f"pr{h}", tag=f"pr{h}")
                    for h in range(NH)] if u else []
            esb = [P.tile([128, max(NU, 1)], F32, name=f"e{h}", tag=f"e{h}")
                   for h in range(NH)] if u else []

            nc.vector.memset(ones1[:], 1.0)
            nc.vector.memset(idones[:], 1.0)
            nc.gpsimd.affine_select(
                ident[:], idones[:], pattern=[[-1, 128]],
                compare_op=mybir.AluOpType.is_equal, fill=0.0,
                base=0, channel_multiplier=1)

            # loads: SP queue carries the QK critical path (wqk cols 0:256
            # then x chunks 0,1,3); the idle Act queue takes wv + x chunk 2.
            def load_x(ch, eng=None):
                sl = slice(ch * 512, (ch + 1) * 512)
                (eng or nc.sync).dma_start(
                    xt[:, :, sl],
                    xT[:, sl].rearrange("(t p) f -> p t f", p=128))
            nc.sync.dma_start(
                wqk[:, :, 0:256],
                wqkT[:, 0:256].rearrange("(t p) f -> p t f", p=128))
            nc.sync.dma_start(
                xt[:, :, 0:256],
                xT[:, 0:256].rearrange("(t p) f -> p t f", p=128))
            nc.sync.dma_start(
                xt[:, :, 256:512],
                xT[:, 256:512].rearrange("(t p) f -> p t f", p=128))
            nc.sync.dma_start(
                xt[:, :, 512:768],
                xT[:, 512:768].rearrange("(t p) f -> p t f", p=128))
            nc.sync.dma_start(wv[:], wvT.rearrange("(t p) f -> p t f", p=128))
            nc.sync.dma_start(
                xt[:, :, 768:1024],
                xT[:, 768:1024].rearrange("(t p) f -> p t f", p=128))
            load_x(2)
            load_x(3)
            nc.sync.dma_start(
                wqk[:, :, 256:384],
                wqkT[:, 256:384].rearrange("(t p) f -> p t f", p=128))
            nc.sync.dma_start(wp01[:], wpT[0:128, :])
            nc.sync.dma_start(wp2[:], wpT[128:192, :])

            qkv_ps = tc.alloc_tile_pool(name="qkv_ps", bufs=1, space="PSUM")
            st_ps = tc.alloc_tile_pool(name="st_ps", bufs=1, space="PSUM")
            av_ps = tc.alloc_tile_pool(name="av_ps", bufs=1, space="PSUM")
            apool = tc.alloc_tile_pool(name="a_sb", bufs=1)
            scr = tc.alloc_tile_pool(name="scr", bufs=1)

            # ---- filler closures (each emits one psum group of work) ----
            def QKg(ch, fi, half=None):
                def f(state):
                    if half is None:
                        sl = slice(ch * 512, (ch + 1) * 512)
                    else:
                        sl = slice(ch * 512 + half * 256,
                                   ch * 512 + half * 256 + 256)
                    w = 512 if half is None else 256
                    dst = (qk0, qk1)[fi]
                    ps = qkv_ps.tile([128, 512], F32, tag="qkvps", bufs=2)
                    for ct in range(CT):
                        nc.tensor.matmul(
                            ps[:, 0:w], wqk[:, ct, fi * 128:(fi + 1) * 128],
                            xt[:, ct, sl],
                            start=(ct == 0), stop=(ct == CT - 1))
                    nc.vector.tensor_copy(dst[:, sl], ps[:, 0:w])
                return f

            def Vg(nt):
                def f(state):
                    ps = qkv_ps.tile([128, 512], F32, tag="qkvps", bufs=2)
                    for ct in range(CT):
                        nc.tensor.matmul(
                            ps[:, 0:NH * D], xt[:, ct, nt * 128:(nt + 1) * 128],
                            wv[:, ct, :],
                            start=(ct == 0), stop=(ct == CT - 1))
                    nc.vector.tensor_copy(
                        vsb[:, nt, :, :],
                        ps[:, 0:NH * D].rearrange("p (h x) -> p h x", x=64))
                return f

            def Q2g(ch):
                def f(state):
                    sl = slice(ch * 512, (ch + 1) * 512)
                    ps = qkv_ps.tile([128, 512], F32, tag="qkvps", bufs=2)
                    for ct in range(CT):
                        nc.tensor.matmul(
                            ps[:], wqk[:, ct, 256:384], xt[:, ct, sl],
                            start=(ct == 0), stop=(ct == CT - 1))
                    nc.vector.tensor_copy(qk2[:, sl], ps[:])
                return f

            def SHIFT(state):
                nc.sync.dma_start(qk2b[64:128, :], qk2[0:64, :])

            def TP(h, qt):
                """Transpose one token-major ao tile to feature-major.

                Uses a qkvps ring buffer: a transpose is a start=True matmul,
                which pending-zeroes its whole psum bank, so it must not share
                the bank where the L sums accumulate."""
                def f(state):
                    src = ao2 if h == 2 else ao01
                    reg = qkv_ps.tile([128, 512], F32, tag="qkvps", bufs=2)
                    nc.tensor.transpose(reg[:, 0:128], src[:, qt, :], ident[:])
                    if h == 2:
                        nc.vector.tensor_copy(
                            aoT2[:, qt * 128:(qt + 1) * 128], reg[0:64, 0:128])
                    else:
                        nc.vector.tensor_copy(
                            aoT01[:, qt * 128:(qt + 1) * 128], reg[:, 0:128])
                return f

            def PJg(co, ch, act_copy=False):
                def f(state):
                    csl = slice(co * 128, (co + 1) * 128)
                    sl = slice(ch * 512, (ch + 1) * 512)
                    ps = qkv_ps.tile([128, 512], F32, tag="qkvps", bufs=2)
                    nc.tensor.matmul(ps[:], wp01[:, csl], aoT01[:, sl],
                                     start=True, stop=False)
                    nc.tensor.matmul(ps[:], wp2[:, csl], aoT2[:, sl],
                                     start=False, stop=True)
                    if act_copy:
                        nc.scalar.activation(otile[co][:, sl], ps[:], CPY)
                    else:
                        nc.vector.tensor_copy(otile[co][:, sl], ps[:])
                return f

            def alloc_state():
                av = av_ps.tile([128, 8, 64], F32, tag="av", bufs=1)
                # lt: cols 0:8 L sums, 8:12 diag scratch (one psum bank).
                lt = av_ps.tile([128, 16], F32, tag="lt", bufs=1)
                return {"av": av, "lt": lt, "pq": [], "a2": None}

            def emit_diag(h, q_ap, k_ap, base, state):
                """Per-token diagonal q.k for the unseen range, exp'd to esb."""
                lt = state["lt"]
                pr = prod[h][base:base + 64, 0:u]
                nc.vector.tensor_tensor(
                    out=pr, in0=q_ap[:, kfull:N], in1=k_ap[:, kfull:N], op=MUL)
                # start=False: these bytes are still pending-zero from the L
                # bank's group start, so the first write stores cleanly, and
                # we must not wipe the accumulated L columns.
                for j in range(NU):
                    nc.tensor.matmul(
                        lt[:, 8 + j:9 + j],
                        prod[h][base:base + 64, j * 128:(j + 1) * 128],
                        ones1[base:base + 64, :], start=False, stop=False,
                        skip_group_check=True)
                nc.scalar.activation(esb[h][:, 0:NU], lt[:, 8:8 + NU],
                                     EXP, scale=0.125)

            AVLAG = 6

            def emit_av_block(h, state):
                t, a = state["pq"].pop(0)
                av, lt = state["av"], state["lt"]
                # start=True pending-zeroes the whole 2KB psum bank, so only
                # the first matmul touching each bank may set it; later
                # regions' first writes land on pending-zero bytes and store
                # (not accumulate) -- exactly what a fresh group needs.
                for qs in range(8):
                    asub = a[:, qs * 128:(qs + 1) * 128]
                    nc.tensor.matmul(
                        av[:, qs, :], asub, vsb[:, t, h, :],
                        start=(t == 0 and qs == 0),
                        stop=(t == T - 1 and qs == 7),
                        skip_group_check=True)
                    nc.tensor.matmul(
                        lt[:, qs:qs + 1], asub, ones1[:],
                        start=(t == 0 and qs == 0),
                        stop=(t == T - 1 and qs == 7),
                        skip_group_check=True)

            pending = {}

            def finish_pending(part=3, norm_split=False):
                """Flush the previous chunk's lagged AV blocks, its diag
                fix, and its normalization, spread over the next chunk's
                first slots: part=0/1 drain AV blocks, part=2 does diag+norm,
                part=3 does everything remaining."""
                if not pending:
                    return
                h, qc, q_ap, k_ap, base, state = pending["v"]
                if part == 0:
                    for _ in range(len(state["pq"]) // 2):
                        emit_av_block(h, state)
                    return
                if part == 1:
                    while state["pq"]:
                        emit_av_block(h, state)
                    return
                del pending["v"]
                while state["pq"]:
                    emit_av_block(h, state)
                if u and qc == 1:
                    emit_diag(h, q_ap, k_ap, base, state)
                emit_norm(h, qc, state, norm_split)

            def emit_attn_chunk(h, q_ap, k_ap, qc, t_lo, t_hi, state,
                                fillers=None, lag=None):
                """ST+exp for key tiles [t_lo, t_hi); AV lags `lag` steps.

                fillers: dict t -> list of closures drained after ST(t)."""
                if lag is None:
                    lag = AVLAG
                for t in range(t_lo, t_hi):
                    st = st_ps.tile([128, 1024], F32, tag="st", bufs=2)
                    for cc in range(2):
                        qsl = slice(qc * 1024 + cc * 512,
                                    qc * 1024 + cc * 512 + 512)
                        nc.tensor.matmul(
                            st[:, cc * 512:(cc + 1) * 512],
                            k_ap[:, t * 128:t * 128 + 128],
                            q_ap[:, qsl], start=True, stop=True)
                    if t == t_lo:
                        finish_pending(0)
                    elif t == t_lo + 1:
                        finish_pending(3)
                    if fillers:
                        for fn in fillers.get(t, ()):
                            fn(state)
                    if len(state["pq"]) > lag:
                        emit_av_block(h, state)
                    a = apool.tile([128, 1024], BF16, tag="a",
                                   bufs=AVLAG + 2)
                    nc.scalar.activation(a[:], st[:], EXP, scale=0.125)
                    state["pq"].append((t, a))

            def emit_norm(h, qc, state, norm_split=False):
                """Denominator fixup + batched reciprocal + normalization."""
                av, lt = state["av"], state["lt"]
                dsttile = ao2 if h == 2 else ao01
                dstoff = 64 if h == 1 else 0
                for qs in range(8):
                    qt = qc * 8 + qs
                    if qt >= ut0:
                        e = esb[h][:, qt - ut0:qt - ut0 + 1]
                        nc.vector.scalar_tensor_tensor(
                            out=av[:, qs, :], in0=vsb[:, qt, h, :],
                            scalar=e, in1=av[:, qs, :], op0=MUL, op1=ADD)
                u_lo = max(ut0 - qc * 8, 0)
                if u_lo < 8:  # add diag exp into the masked-tile L sums
                    nc.vector.tensor_tensor(
                        out=lt[:, u_lo:8], in0=lt[:, u_lo:8],
                        in1=esb[h][:, qc * 8 + u_lo - ut0:qc * 8 + 8 - ut0],
                        op=ADD)
                rec = scr.tile([128, 8], F32, tag="rec", bufs=2)
                nc.vector.reciprocal(rec[:], lt[:, 0:8])
                for qs in range(8):
                    qt = qc * 8 + qs
                    dst = dsttile[:, qt, dstoff:dstoff + 64]
                    if norm_split and qs % 2:
                        nc.scalar.activation(dst, av[:, qs, :], CPY,
                                             scale=rec[:, qs:qs + 1])
                    else:
                        nc.vector.tensor_scalar_mul(
                            dst, av[:, qs, :], rec[:, qs:qs + 1])

            # ---- filler schedules (tuned for T == 12; fallback: upfront) ----
            fill = {(h, qc): {} for h in range(NH) for qc in range(2)}
            flushf = {(h, qc): [] for h in range(NH) for qc in range(2)}

            def put(h, qc, t, fn):
                fill[(h, qc)].setdefault(t, []).append(fn)

            if T == 12:
                lead = [QKg(0, 0, 0), QKg(0, 0, 1), QKg(0, 1, 0),
                        QKg(0, 1, 1), QKg(1, 0, 0), QKg(1, 0, 1)]
                # x1 halves land around the last two lead groups
                sched = {0: [Vg(0), Vg(1)], 1: [QKg(1, 1), Vg(2)],
                         2: [Vg(3)], 3: [Vg(4)], 4: [Vg(5)], 5: [Vg(6)],
                         6: [Vg(7)], 7: [QKg(2, 1), Vg(8)],
                         8: [QKg(2, 0), Vg(9)], 9: [QKg(3, 0), Vg(10)],
                         10: [Vg(11)]}
                for t, fns in sched.items():
                    for fn in fns:
                        put(0, 0, t, fn)
                put(0, 1, 8, QKg(3, 1))
                for i in range(4):
                    put(0, 1, 2 + i, Vg(12 + i))
                for ch in range(NCH):
                    put(1, 0, 2 + ch, Q2g(ch))
                put(1, 0, 6, SHIFT)
                # transposes read the previous chunk's normalized ao, which
                # lands at slot 1 (finish_pending) -- schedule them from slot 2
                for i in range(8):
                    put(1, 1, 2 + i, TP(0, i))      # pair tiles of qc0
                for i in range(8):
                    put(2, 0, 2 + i, TP(0, 8 + i))  # pair tiles of qc1
                for i in range(8):
                    put(2, 1, 2 + i, TP(2, i))      # head-2 tiles of qc0
                for i in range(CT):
                    put(2, 1, 6 + i, PJg(i, 0))
                put(2, 1, 10, PJg(0, 1))
                put(2, 1, 11, PJg(1, 1))
                pj_tail = None
            else:
                lead = ([QKg(ch, fi) for ch in range(NCH) for fi in (0, 1)]
                        + [Vg(nt) for nt in range(QT)]
                        + [Q2g(ch) for ch in range(NCH)] + [SHIFT])
                pj_tail = [(co, ch, co % 2 == 1)
                           for ch in range(NCH) for co in range(CT)]

            # ---- emission ----
            # dummy matmuls ramp the PE p-state during the initial DMA wait
            # (full clock needs 3us of continuous PE busy); they overlap the
            # x/wqk transfers and abut the first real projection group.
            NDUM = int(os.environ.get("KERNEL_NDUM", "8"))
            if NDUM:
                dmy = qkv_ps.tile([128, 512], F32, tag="qkvps", bufs=2)
                for _ in range(NDUM):
                    nc.tensor.matmul(dmy[0:1, 0:128], idones[0:1, 0:1],
                                     idones[0:1, 0:128], start=True, stop=True)
            for fn in lead:
                fn(None)

            heads = [(qk0[0:64, :], qk1[0:64, :], 0),
                     (qk0[64:128, :], qk1[64:128, :], 64),
                     (qk2b[64:128, :], qk2[64:128, :], 64)]

            for h, (q_ap, k_ap, base) in enumerate(heads):
                for qc in range(2):
                    state = alloc_state()
                    emit_attn_chunk(h, q_ap, k_ap, qc, 0, T, state,
                                    fill[(h, qc)],
                                    lag=3 if (h, qc) == (2, 1) else None)
                    pending["v"] = (h, qc, q_ap, k_ap, base, state)
                    if T != 12:
                        finish_pending()
                        # fallback: transpose finished tiles in place
                        if h == 1:
                            for i in range(8):
                                TP(0, qc * 8 + i)(state)
                        elif h == 2:
                            for i in range(8):
                                TP(2, qc * 8 + i)(state)
            finish_pending(norm_split=True)

            for _pool in (scr, apool, av_ps, st_ps, qkv_ps):
                _pool.release()

            # ---- tail: head-2 qc1 transposes + remaining projection ----
            with tc.tile_pool(name="tp_ps", bufs=1, space="PSUM") as tpps, \
                 tc.tile_pool(name="pj_ps", bufs=1, space="PSUM") as pjps, \
                 tc.tile_pool(name="ost", bufs=1) as ost:
                def tail_tp(qt):
                    reg = tpps.tile([128, 128], F32, tag="tp", bufs=2)
                    nc.tensor.transpose(reg[:], ao2[:, qt, :], ident[:])
                    if qt % 2:
                        nc.scalar.activation(
                            aoT2[:, qt * 128:(qt + 1) * 128], reg[0:64, :], CPY)
                    else:
                        nc.vector.tensor_copy(
                            aoT2[:, qt * 128:(qt + 1) * 128], reg[0:64, :])

                def tail_pj(co, ch, act):
                    csl = slice(co * 128, (co + 1) * 128)
                    sl = slice(ch * 512, (ch + 1) * 512)
                    ps = pjps.tile([128, 512], F32, tag="pj", bufs=3)
                    nc.tensor.matmul(ps[:], wp01[:, csl], aoT01[:, sl],
                                     start=True, stop=False)
                    nc.tensor.matmul(ps[:], wp2[:, csl], aoT2[:, sl],
                                     start=False, stop=True)
                    if act:
                        nc.scalar.activation(otile[co][:, sl], ps[:], CPY)
                    else:
                        nc.vector.tensor_copy(otile[co][:, sl], ps[:])

                if T == 12:
                    for co in range(2, CT):
                        tail_pj(co, 1, co % 2 == 0)
                    for qt in range(8, 16):
                        tail_tp(qt)
                    nalt = 0
                    qeng = [nc.sync, nc.scalar, nc.gpsimd]
                    for co in range(CT):
                        for ch in (2, 3):
                            tail_pj(co, ch, nalt % 2 == 0)
                            nalt += 1
                        csl = slice(co * 128, (co + 1) * 128)
                        if co == CT - 1:
                            nc.sync.dma_start(yT[csl, 0:1024],
                                              otile[co][:, 0:1024])
                            nc.scalar.dma_start(yT[csl, 1024:2048],
                                                otile[co][:, 1024:2048])
                        else:
                            qeng[co % 3].dma_start(yT[csl, :], otile[co][:])
                else:
                    for co, ch, act in pj_tail:
                        tail_pj(co, ch, act)
                    for co in range(CT):
                        csl = slice(co * 128, (co + 1) * 128)
                        nc.sync.dma_start(yT[csl, :], otile[co][:])

    nc.compile()
    return nc


def kernel(**inputs):
    global _last_results
    from concourse.bass_utils import run_bass_kernel_spmd

    x = np.asarray(inputs["x"], np.float32)
    w_qkv = np.asarray(inputs["w_qkv"], np.float32)
    w_proj = np.asarray(inputs["w_proj"], np.float32)
    b_proj = np.asarray(inputs["b_proj"], np.float32)
    u = int(np.asarray(inputs["unseen_size"]))
    B = x.shape[0]

    nc = _build(u)

    wT = np.ascontiguousarray(w_qkv.T).astype(NPBF)        # [768, 2304]
    wpT_full = np.ascontiguousarray(w_proj.T).astype(NPBF)  # [768(ci), 768(co)]
    xTb = [np.ascontiguousarray(x[b].T).astype(NPBF) for b in range(B)]

    in_maps = []
    for core in range(8):
        b, g = divmod(core, 4)
        hs = [3 * g, 3 * g + 1, 3 * g + 2]
        cols = []
        for h in hs[:2]:
            cols += [0 * C + h * D + i for i in range(D)]   # q0 q1
        for h in hs[:2]:
            cols += [1 * C + h * D + i for i in range(D)]   # k0 k1
        cols += [0 * C + hs[2] * D + i for i in range(D)]   # q2
        cols += [1 * C + hs[2] * D + i for i in range(D)]   # k2
        wqkT = np.ascontiguousarray(wT[:, cols])
        vcols = [2 * C + h * D + i for h in hs for i in range(D)]
        wvT = np.ascontiguousarray(wT[:, vcols])
        ci = [h * D + i for h in hs for i in range(D)]
        wpT = np.ascontiguousarray(wpT_full[ci, :])
        in_maps.append({"xT": xTb[b], "wqkT": wqkT, "wvT": wvT, "wpT": wpT})

    trace = bool(int(os.environ.get("KERNEL_TRACE", "0")))
    res = run_bass_kernel_spmd(nc, in_maps, core_ids=list(range(8)), trace=trace)
    _last_results = res

    y = np.zeros((B, N, C), np.float32)
    for core in range(8):
        b = core // 4
        y[b] += np.asarray(res.results[core]["yT"], np.float32).T
    y += b_proj
    return y


# revision 8
# speedup vs baseline: 1.0879x; 1.0025x over previous
"""MCCDecoderAttention Trainium2 kernel (8 NeuronCores), v3.

Sharding: core = b*4 + g  (b in {0,1} batch, g in {0..3} head-group).
Each core computes attention for 3 heads of one batch plus its partial
output projection; the host sums 4 partials per batch and adds b_proj.

Layout (all matmul operands bf16, f32 PSUM accumulation):
  - Q/K projected feature-major into packed 128-row tiles
    qk0=[q0;q1], qk1=[k0;k1], qk2=[q2;k2]; q2 is copied to partitions
    64:128 of qk2b by an SBUF->SBUF DMA so head 2's matmuls have
    matching base partitions.
  - V projected token-major straight into vsb[key, head, 0:64].
  - Scores S^T computed per (head, 1024-query chunk, 128-key tile);
    exp on ScalarE (scale=1/8 folded, no max subtraction needed).
  - AV uses the transposed form: lhsT = A^T subtile [128k x 128q],
    rhs = vsb [128k x 64] -> psum [128q, 64] accumulated over key
    tiles; the softmax denominator L accumulates in parallel via
    1-column ones matmuls.  The narrow free dims halve PE cost vs the
    feature-major AV form.
  - Decoder mask: keys limited to [0, N-u); each unseen query gets its
    diagonal term back via a 1-column ones-matmul of q*k (elementwise),
    exp, and a fused DVE multiply-add into the psum accumulator.
  - Normalized ao is written token-major f32 with heads 0/1 interleaved
    per query tile; PE transposes [128q x 128c] tiles (into spare
    columns of the L psum bank) flip it feature-major for the output
    projection, which contracts [128ci]+[64ci] per psum group.

Scheduling: emission is software-pipelined (ST(t) is emitted before the
AV block of t-1 so the PE never queues behind the exp dependency), and
all projection / transpose / phase-3 work is drained as per-key-tile
"fillers" inside the Act-bound attention loops.
"""

import functools
import os
import sys

for _p in ("/opt/trn_rl_repo", "/root/.axon_site/_ro/trn_rl_repo"):
    if os.path.isdir(_p) and _p not in sys.path:
        sys.path.insert(0, _p)

import numpy as np

import concourse.bacc as bacc
import concourse.tile as tile
from concourse import mybir

N, C, D = 2048, 768, 64
NH = 3            # heads per core
CT = C // 128     # 6 contraction tiles
F32 = mybir.dt.float32
BF16 = mybir.dt.bfloat16
FP8 = mybir.dt.float8e4
DR = mybir.MatmulPerfMode.DoubleRow
NPBF = mybir.dt.np(BF16)
EXP = mybir.ActivationFunctionType.Exp
CPY = mybir.ActivationFunctionType.Copy
MUL = mybir.AluOpType.mult
ADD = mybir.AluOpType.add

_last_results = None  # BassKernelResults of the most recent run (for test.py)


@functools.lru_cache(maxsize=4)
def _build(u: int):
    assert u % 128 == 0 and 0 <= u <= 512, f"unsupported unseen_size {u}"
    nc = bacc.Bacc(None, target_bir_lowering=False)
    xT = nc.dram_tensor("xT", [C, N], BF16, kind="ExternalInput")
    wqkT = nc.dram_tensor("wqkT", [C, 6 * D], BF16, kind="ExternalInput")
    wvT = nc.dram_tensor("wvT", [C, NH * D], BF16, kind="ExternalInput")
    wpT = nc.dram_tensor("wpT", [NH * D, C], BF16, kind="ExternalInput")
    yT = nc.dram_tensor("yT", [C, N], BF16, kind="ExternalOutput")

    kfull = N - u
    T = (kfull + 127) // 128           # key tiles
    QT = N // 128                      # 16 query tiles
    ut0 = kfull // 128                 # first unseen query tile
    NCH = N // 512                     # 4 token chunks
    NU = u // 128                      # unseen tiles

    with nc.allow_low_precision(reason="bf16 staging/outputs"), \
         tile.TileContext(nc) as tc:
        with tc.tile_pool(name="persist", bufs=1) as P:
            xt = P.tile([128, CT, N], BF16)
            wqk = P.tile([128, CT, 6 * D], BF16)
            wv = P.tile([128, CT, NH * D], BF16)
            wp01 = P.tile([128, C], BF16)
            wp2 = P.tile([64, C], BF16)
            qk0 = P.tile([128, N], BF16)   # [q0; q1]
            qk1 = P.tile([128, N], BF16)   # [k0; k1]
            qk2 = P.tile([128, N], BF16)   # [q2; k2]
            qk2b = P.tile([128, N], BF16)  # [-; q2] (dma partition shift)
            vsb = P.tile([128, QT, NH, 64], BF16)
            # token-major normalized attention out: heads 0/1 interleaved per
            # query tile; head 2 in the low half of its own tile (high half
            # junk, transposed but never copied out).
            ao01 = P.tile([128, QT, 128], F32)
            ao2 = P.tile([128, QT, 128], F32)
            aoT01 = P.tile([128, N], BF16)
            aoT2 = P.tile([64, N], BF16)
            otile = [P.tile([128, N], BF16, name=f"ot{co}", tag=f"ot{co}")
                     for co in range(CT)]
            ones1 = P.tile([128, 1], BF16)
            ident = P.tile([128, 128], F32)
            idones = P.tile([128, 128], F32)
            prod = [P.tile([128, max(u, 1)], BF16, name=f"pr{h}", tag=f"pr{h}")
                    for h in range(NH)] if u else []
            esb = [P.tile([128, max(NU, 1)], F32, name=f"e{h}", tag=f"e{h}")
                   for h in range(NH)] if u else []

            nc.vector.memset(ones1[:], 1.0)
            nc.vector.memset(idones[:], 1.0)
            nc.gpsimd.affine_select(
                ident[:], idones[:], pattern=[[-1, 128]],
                compare_op=mybir.AluOpType.is_equal, fill=0.0,
                base=0, channel_multiplier=1)

            # loads: SP queue carries the QK critical path (wqk cols 0:256
            # then x chunks 0,1,3); the idle Act queue takes wv + x chunk 2.
            def load_x(ch, eng=None):
                sl = slice(ch * 512, (ch + 1) * 512)
                (eng or nc.sync).dma_start(
                    xt[:, :, sl],
                    xT[:, sl].rearrange("(t p) f -> p t f", p=128))
            nc.sync.dma_start(
                wqk[:, :, 0:256],
                wqkT[:, 0:256].rearrange("(t p) f -> p t f", p=128))
            nc.sync.dma_start(
                xt[:, :, 0:256],
                xT[:, 0:256].rearrange("(t p) f -> p t f", p=128))
            nc.sync.dma_start(
                xt[:, :, 256:512],
                xT[:, 256:512].rearrange("(t p) f -> p t f", p=128))
            nc.sync.dma_start(
                xt[:, :, 512:768],
                xT[:, 512:768].rearrange("(t p) f -> p t f", p=128))
            nc.sync.dma_start(wv[:], wvT.rearrange("(t p) f -> p t f", p=128))
            nc.sync.dma_start(
                xt[:, :, 768:1024],
                xT[:, 768:1024].rearrange("(t p) f -> p t f", p=128))
            load_x(2)
            load_x(3)
            nc.sync.dma_start(
                wqk[:, :, 256:384],
                wqkT[:, 256:384].rearrange("(t p) f -> p t f", p=128))
            nc.sync.dma_start(wp01[:], wpT[0:128, :])
            nc.sync.dma_start(wp2[:], wpT[128:192, :])

            qkv_ps = tc.alloc_tile_pool(name="qkv_ps", bufs=1, space="PSUM")
            st_ps = tc.alloc_tile_pool(name="st_ps", bufs=1, space="PSUM")
            av_ps = tc.alloc_tile_pool(name="av_ps", bufs=1, space="PSUM")
            apool = tc.alloc_tile_pool(name="a_sb", bufs=1)
            scr = tc.alloc_tile_pool(name="scr", bufs=1)

            # ---- filler closures (each emits one psum group of work) ----
            def QKg(ch, fi, half=None):
                def f(state):
                    if half is None:
                        sl = slice(ch * 512, (ch + 1) * 512)
                    else:
                        sl = slice(ch * 512 + half * 256,
                                   ch * 512 + half * 256 + 256)
                    w = 512 if half is None else 256
                    dst = (qk0, qk1)[fi]
                    ps = qkv_ps.tile([128, 512], F32, tag="qkvps", bufs=2)
                    for ct in range(CT):
                        nc.tensor.matmul(
                            ps[:, 0:w], wqk[:, ct, fi * 128:(fi + 1) * 128],
                            xt[:, ct, sl],
                            start=(ct == 0), stop=(ct == CT - 1))
                    nc.vector.tensor_copy(dst[:, sl], ps[:, 0:w])
                return f

            def Vg(nt):
                def f(state):
                    ps = qkv_ps.tile([128, 512], F32, tag="qkvps", bufs=2)
                    for ct in range(CT):
                        nc.tensor.matmul(
                            ps[:, 0:NH * D], xt[:, ct, nt * 128:(nt + 1) * 128],
                            wv[:, ct, :],
                            start=(ct == 0), stop=(ct == CT - 1))
                    nc.vector.tensor_copy(
                        vsb[:, nt, :, :],
                        ps[:, 0:NH * D].rearrange("p (h x) -> p h x", x=64))
                return f

            def Q2g(ch):
                def f(state):
                    sl = slice(ch * 512, (ch + 1) * 512)
                    ps = qkv_ps.tile([128, 512], F32, tag="qkvps", bufs=2)
                    for ct in range(CT):
                        nc.tensor.matmul(
                            ps[:], wqk[:, ct, 256:384], xt[:, ct, sl],
                            start=(ct == 0), stop=(ct == CT - 1))
                    nc.vector.tensor_copy(qk2[:, sl], ps[:])
                return f

            def SHIFT(state):
                nc.sync.dma_start(qk2b[64:128, :], qk2[0:64, :])

            def TP(h, qt):
                """Transpose one token-major ao tile to feature-major.

                Uses a qkvps ring buffer: a transpose is a start=True matmul,
                which pending-zeroes its whole psum bank, so it must not share
                the bank where the L sums accumulate."""
                def f(state):
                    src = ao2 if h == 2 else ao01
                    reg = qkv_ps.tile([128, 512], F32, tag="qkvps", bufs=2)
                    nc.tensor.transpose(reg[:, 0:128], src[:, qt, :], ident[:])
                    if h == 2:
                        nc.vector.tensor_copy(
                            aoT2[:, qt * 128:(qt + 1) * 128], reg[0:64, 0:128])
                    else:
                        nc.vector.tensor_copy(
                            aoT01[:, qt * 128:(qt + 1) * 128], reg[:, 0:128])
                return f

            def PJg(co, ch, act_copy=False):
                def f(state):
                    csl = slice(co * 128, (co + 1) * 128)
                    sl = slice(ch * 512, (ch + 1) * 512)
                    ps = qkv_ps.tile([128, 512], F32, tag="qkvps", bufs=2)
                    nc.tensor.matmul(ps[:], wp01[:, csl], aoT01[:, sl],
                                     start=True, stop=False)
                    nc.tensor.matmul(ps[:], wp2[:, csl], aoT2[:, sl],
                                     start=False, stop=True)
                    if act_copy:
                        nc.scalar.activation(otile[co][:, sl], ps[:], CPY)
                    else:
                        nc.vector.tensor_copy(otile[co][:, sl], ps[:])
                return f

            def alloc_state():
                av = av_ps.tile([128, 8, 64], F32, tag="av", bufs=1)
                # lt: cols 0:8 L sums, 8:12 diag scratch (one psum bank).
                lt = av_ps.tile([128, 16], F32, tag="lt", bufs=1)
                return {"av": av, "lt": lt, "pq": [], "a2": None}

            def emit_diag(h, q_ap, k_ap, base, state):
                """Per-token diagonal q.k for the unseen range, exp'd to esb."""
                lt = state["lt"]
                pr = prod[h][base:base + 64, 0:u]
                nc.vector.tensor_tensor(
                    out=pr, in0=q_ap[:, kfull:N], in1=k_ap[:, kfull:N], op=MUL)
                # start=False: these bytes are still pending-zero from the L
                # bank's group start, so the first write stores cleanly, and
                # we must not wipe the accumulated L columns.
                for j in range(NU):
                    nc.tensor.matmul(
                        lt[:, 8 + j:9 + j],
                        prod[h][base:base + 64, j * 128:(j + 1) * 128],
                        ones1[base:base + 64, :], start=False, stop=False,
                        skip_group_check=True)
                nc.scalar.activation(esb[h][:, 0:NU], lt[:, 8:8 + NU],
                                     EXP, scale=0.125)

            AVLAG = 6

            def emit_av_block(h, state):
                t, a = state["pq"].pop(0)
                av, lt = state["av"], state["lt"]
                # start=True pending-zeroes the whole 2KB psum bank, so only
                # the first matmul touching each bank may set it; later
                # regions' first writes land on pending-zero bytes and store
                # (not accumulate) -- exactly what a fresh group needs.
                for qs in range(8):
                    asub = a[:, qs * 128:(qs + 1) * 128]
                    nc.tensor.matmul(
                        av[:, qs, :], asub, vsb[:, t, h, :],
                        start=(t == 0 and qs == 0),
                        stop=(t == T - 1 and qs == 7),
                        skip_group_check=True)
                    nc.tensor.matmul(
                        lt[:, qs:qs + 1], asub, ones1[:],
                        start=(t == 0 and qs == 0),
                        stop=(t == T - 1 and qs == 7),
                        skip_group_check=True)

            pending = {}

            def finish_pending(part=3, norm_split=False):
                """Flush the previous chunk's lagged AV blocks, its diag
                fix, and its normalization, spread over the next chunk's
                first slots: part=0/1 drain AV blocks, part=2 does diag+norm,
                part=3 does everything remaining."""
                if not pending:
                    return
                h, qc, q_ap, k_ap, base, state = pending["v"]
                if part == 0:
                    for _ in range(len(state["pq"]) // 2):
                        emit_av_block(h, state)
                    return
                if part == 1:
                    while state["pq"]:
                        emit_av_block(h, state)
                    return
                del pending["v"]
                while state["pq"]:
                    emit_av_block(h, state)
                if u and qc == 1:
                    emit_diag(h, q_ap, k_ap, base, state)
                emit_norm(h, qc, state, norm_split)

            def emit_attn_chunk(h, q_ap, k_ap, qc, t_lo, t_hi, state,
                                fillers=None, lag=None):
                """ST+exp for key tiles [t_lo, t_hi); AV lags `lag` steps.

                fillers: dict t -> list of closures drained after ST(t)."""
                if lag is None:
                    lag = AVLAG
                for t in range(t_lo, t_hi):
                    st = st_ps.tile([128, 1024], F32, tag="st", bufs=2)
                    for cc in range(2):
                        qsl = slice(qc * 1024 + cc * 512,
                                    qc * 1024 + cc * 512 + 512)
                        nc.tensor.matmul(
                            st[:, cc * 512:(cc + 1) * 512],
                            k_ap[:, t * 128:t * 128 + 128],
                            q_ap[:, qsl], start=True, stop=True)
                    if t == t_lo:
                        finish_pending(0)
                    elif t == t_lo + 1:
                        finish_pending(3)
                    if fillers:
                        for fn in fillers.get(t, ()):
                            fn(state)
                    if len(state["pq"]) > lag:
                        emit_av_block(h, state)
                    a = apool.tile([128, 1024], BF16, tag="a",
                                   bufs=AVLAG + 2)
                    nc.scalar.activation(a[:], st[:], EXP, scale=0.125)
                    state["pq"].append((t, a))

            def emit_norm(h, qc, state, norm_split=False):
                """Denominator fixup + batched reciprocal + normalization."""
                av, lt = state["av"], state["lt"]
                dsttile = ao2 if h == 2 else ao01
                dstoff = 64 if h == 1 else 0
                for qs in range(8):
                    qt = qc * 8 + qs
                    if qt >= ut0:
                        e = esb[h][:, qt - ut0:qt - ut0 + 1]
                        nc.vector.scalar_tensor_tensor(
                            out=av[:, qs, :], in0=vsb[:, qt, h, :],
                            scalar=e, in1=av[:, qs, :], op0=MUL, op1=ADD)
                u_lo = max(ut0 - qc * 8, 0)
                if u_lo < 8:  # add diag exp into the masked-tile L sums
                    nc.vector.tensor_tensor(
                        out=lt[:, u_lo:8], in0=lt[:, u_lo:8],
                        in1=esb[h][:, qc * 8 + u_lo - ut0:qc * 8 + 8 - ut0],
                        op=ADD)
                rec = scr.tile([128, 8], F32, tag="rec", bufs=2)
                nc.vector.reciprocal(rec[:], lt[:, 0:8])
                for qs in range(8):
                    qt = qc * 8 + qs
                    dst = dsttile[:, qt, dstoff:dstoff + 64]
                    if norm_split and qs % 2:
                        nc.scalar.activation(dst, av[:, qs, :], CPY,
                                             scale=rec[:, qs:qs + 1])
                    else:
                        nc.vector.tensor_scalar_mul(
                            dst, av[:, qs, :], rec[:, qs:qs + 1])

            # ---- filler schedules (tuned for T == 12; fallback: upfront) ----
            fill = {(h, qc): {} for h in range(NH) for qc in range(2)}
            flushf = {(h, qc): [] for h in range(NH) for qc in range(2)}

            def put(h, qc, t, fn):
                fill[(h, qc)].setdefault(t, []).append(fn)

            if T == 12:
                lead = [QKg(0, 0, 0), QKg(0, 0, 1), QKg(0, 1, 0),
                        QKg(0, 1, 1), QKg(1, 0, 0), QKg(1, 0, 1)]
                # x1 halves land around the last two lead groups
                sched = {0: [Vg(0), Vg(1)], 1: [QKg(1, 1), Vg(2)],
                         2: [Vg(3)], 3: [Vg(4)], 4: [Vg(5)], 5: [Vg(6)],
                         6: [Vg(7)], 7: [QKg(2, 1), Vg(8)],
                         8: [QKg(2, 0), Vg(9)], 9: [QKg(3, 0), Vg(10)],
                         10: [Vg(11)]}
                for t, fns in sched.items():
                    for fn in fns:
                        put(0, 0, t, fn)
                put(0, 1, 8, QKg(3, 1))
                for i in range(4):
                    put(0, 1, 2 + i, Vg(12 + i))
                for ch in range(NCH):
                    put(1, 0, 2 + ch, Q2g(ch))
                put(1, 0, 6, SHIFT)
                # transposes read the previous chunk's normalized ao, which
                # lands at slot 1 (finish_pending) -- schedule them from slot 2
                for i in range(8):
                    put(1, 1, 2 + i, TP(0, i))      # pair tiles of qc0
                for i in range(8):
                    put(2, 0, 2 + i, TP(0, 8 + i))  # pair tiles of qc1
                for i in range(8):
                    put(2, 1, 2 + i, TP(2, i))      # head-2 tiles of qc0
                for i in range(CT):
                    put(2, 1, 6 + i, PJg(i, 0))
                put(2, 1, 10, PJg(0, 1))
                put(2, 1, 11, PJg(1, 1))
                pj_tail = None
            else:
                lead = ([QKg(ch, fi) for ch in range(NCH) for fi in (0, 1)]
                        + [Vg(nt) for nt in range(QT)]
                        + [Q2g(ch) for ch in range(NCH)] + [SHIFT])
                pj_tail = [(co, ch, co % 2 == 1)
                           for ch in range(NCH) for co in range(CT)]

            # ---- emission ----
            # dummy matmuls ramp the PE p-state during the initial DMA wait
            # (full clock needs 3us of continuous PE busy); they overlap the
            # x/wqk transfers and abut the first real projection group.
            NDUM = int(os.environ.get("KERNEL_NDUM", "8"))
            if NDUM:
                dmy = qkv_ps.tile([128, 512], F32, tag="qkvps", bufs=2)
                for _ in range(NDUM):
                    nc.tensor.matmul(dmy[0:1, 0:128], idones[0:1, 0:1],
                                     idones[0:1, 0:128], start=True, stop=True)
            for fn in lead:
                fn(None)

            heads = [(qk0[0:64, :], qk1[0:64, :], 0),
                     (qk0[64:128, :], qk1[64:128, :], 64),
                     (qk2b[64:128, :], qk2[64:128, :], 64)]

            for h, (q_ap, k_ap, base) in enumerate(heads):
                for qc in range(2):
                    state = alloc_state()
                    emit_attn_chunk(h, q_ap, k_ap, qc, 0, T, state,
                                    fill[(h, qc)],
                                    lag=3 if (h, qc) == (2, 1) else None)
                    pending["v"] = (h, qc, q_ap, k_ap, base, state)
                    if T != 12:
                        finish_pending()
                        # fallback: transpose finished tiles in place
                        if h == 1:
                            for i in range(8):
                                TP(0, qc * 8 + i)(state)
                        elif h == 2:
                            for i in range(8):
                                TP(2, qc * 8 + i)(state)
            finish_pending(norm_split=True)

            for _pool in (scr, apool, av_ps, st_ps, qkv_ps):
                _pool.release()

            # ---- tail: head-2 qc1 transposes + remaining projection ----
            with tc.tile_pool(name="tp_ps", bufs=1, space="PSUM") as tpps, \
                 tc.tile_pool(name="pj_ps", bufs=1, space="PSUM") as pjps, \
                 tc.tile_pool(name="ost", bufs=1) as ost:
                def tail_tp(qt):
                    reg = tpps.tile([128, 128], F32, tag="tp", bufs=2)
                    nc.tensor.transpose(reg[:], ao2[:, qt, :], ident[:])
                    if qt % 2:
                        nc.scalar.activation(
                            aoT2[:, qt * 128:(qt + 1) * 128], reg[0:64, :], CPY)
                    else:
                        nc.vector.tensor_copy(
                            aoT2[:, qt * 128:(qt + 1) * 128], reg[0:64, :])

                def tail_pj(co, ch, act):
                    csl = slice(co * 128, (co + 1) * 128)
                    sl = slice(ch * 512, (ch + 1) * 512)
                    ps = pjps.tile([128, 512], F32, tag="pj", bufs=3)
                    nc.tensor.matmul(ps[:], wp01[:, csl], aoT01[:, sl],
                                     start=True, stop=False)
                    nc.tensor.matmul(ps[:], wp2[:, csl], aoT2[:, sl],
                                     start=False, stop=True)
                    if act:
                        nc.scalar.activation(otile[co][:, sl], ps[:], CPY)
                    else:
                        nc.vector.tensor_copy(otile[co][:, sl], ps[:])

                if T == 12:
                    for co in range(2, CT):
                        tail_pj(co, 1, co % 2 == 0)
                    for qt in range(8, 16):
                        tail_tp(qt)
                    qeng = [nc.sync, nc.scalar, nc.gpsimd]
                    for co in range(CT):
                        # first half (chunks 0+1) is already staged: drain it
                        # now so the shared DMA device starts early
                        csl = slice(co * 128, (co + 1) * 128)
                        qeng[co % 3].dma_start(yT[csl, 0:1024],
                                               otile[co][:, 0:1024])
                    nalt = 0
                    for co in range(CT):
                        for ch in (2, 3):
                            tail_pj(co, ch, nalt % 2 == 0)
                            nalt += 1
                        csl = slice(co * 128, (co + 1) * 128)
                        qeng[co % 3].dma_start(yT[csl, 1024:2048],
                                               otile[co][:, 1024:2048])
                else:
                    for co, ch, act in pj_tail:
                        tail_pj(co, ch, act)
                    for co in range(CT):
                        csl = slice(co * 128, (co + 1) * 128)
                        nc.sync.dma_start(yT[csl, :], otile[co][:])

    nc.compile()
    return nc


def kernel(**inputs):
    global _last_results
    from concourse.bass_utils import run_bass_kernel_spmd

    x = np.asarray(inputs["x"], np.float32)
    w_qkv = np.asarray(inputs["w_qkv"], np.float32)
    w_proj = np.asarray(inputs["w_proj"], np.float32)
    b_proj = np.asarray(inputs["b_proj"], np.float32)
    u = int(np.asarray(inputs["unseen_size"]))
    B = x.shape[0]

    nc = _build(u)

    wT = np.ascontiguousarray(w_qkv.T).astype(NPBF)        # [768, 2304]
    wpT_full = np.ascontiguousarray(w_proj.T).astype(NPBF)  # [768(ci), 768(co)]
    xTb = [np.ascontiguousarray(x[b].T).astype(NPBF) for b in range(B)]

    in_maps = []
    for core in range(8):
        b, g = divmod(core, 4)
        hs = [3 * g, 3 * g + 1, 3 * g + 2]
        cols = []
        for h in hs[:2]:
            cols += [0 * C + h * D + i for i in range(D)]   # q0 q1
        for h in hs[:2]:
            cols += [1 * C + h * D + i for i in range(D)]   # k0 k1
        cols += [0 * C + hs[2] * D + i for i in range(D)]   # q2
        cols += [1 * C + hs[2] * D + i for i in range(D)]   # k2
        wqkT = np.ascontiguousarray(wT[:, cols])
        vcols = [2 * C + h * D + i for h in hs for i in range(D)]
        wvT = np.ascontiguousarray(wT[:, vcols])
        ci = [h * D + i for h in hs for i in range(D)]
        wpT = np.ascontiguousarray(wpT_full[ci, :])
        in_maps.append({"xT": xTb[b], "wqkT": wqkT, "wvT": wvT, "wpT": wpT})

    trace = bool(int(os.environ.get("KERNEL_TRACE", "0")))
    res = run_bass_kernel_spmd(nc, in_maps, core_ids=list(range(8)), trace=trace)
    _last_results = res

    y = np.zeros((B, N, C), np.float32)
    for core in range(8):
        b = core // 4
        y[b] += np.asarray(res.results[core]["yT"], np.float32).T
    y += b_proj
    return y


# revision 9
# speedup vs baseline: 1.0931x; 1.0048x over previous
"""MCCDecoderAttention Trainium2 kernel (8 NeuronCores), v3.

Sharding: core = b*4 + g  (b in {0,1} batch, g in {0..3} head-group).
Each core computes attention for 3 heads of one batch plus its partial
output projection; the host sums 4 partials per batch and adds b_proj.

Layout (all matmul operands bf16, f32 PSUM accumulation):
  - Q/K projected feature-major into packed 128-row tiles
    qk0=[q0;q1], qk1=[k0;k1], qk2=[q2;k2]; q2 is copied to partitions
    64:128 of qk2b by an SBUF->SBUF DMA so head 2's matmuls have
    matching base partitions.
  - V projected token-major straight into vsb[key, head, 0:64].
  - Scores S^T computed per (head, 1024-query chunk, 128-key tile);
    exp on ScalarE (scale=1/8 folded, no max subtraction needed).
  - AV uses the transposed form: lhsT = A^T subtile [128k x 128q],
    rhs = vsb [128k x 64] -> psum [128q, 64] accumulated over key
    tiles; the softmax denominator L accumulates in parallel via
    1-column ones matmuls.  The narrow free dims halve PE cost vs the
    feature-major AV form.
  - Decoder mask: keys limited to [0, N-u); each unseen query gets its
    diagonal term back via a 1-column ones-matmul of q*k (elementwise),
    exp, and a fused DVE multiply-add into the psum accumulator.
  - Normalized ao is written token-major f32 with heads 0/1 interleaved
    per query tile; PE transposes [128q x 128c] tiles (into spare
    columns of the L psum bank) flip it feature-major for the output
    projection, which contracts [128ci]+[64ci] per psum group.

Scheduling: emission is software-pipelined (ST(t) is emitted before the
AV block of t-1 so the PE never queues behind the exp dependency), and
all projection / transpose / phase-3 work is drained as per-key-tile
"fillers" inside the Act-bound attention loops.
"""

import functools
import os
import sys

for _p in ("/opt/trn_rl_repo", "/root/.axon_site/_ro/trn_rl_repo"):
    if os.path.isdir(_p) and _p not in sys.path:
        sys.path.insert(0, _p)

import numpy as np

import concourse.bacc as bacc
import concourse.tile as tile
from concourse import mybir

N, C, D = 2048, 768, 64
NH = 3            # heads per core
CT = C // 128     # 6 contraction tiles
F32 = mybir.dt.float32
BF16 = mybir.dt.bfloat16
FP8 = mybir.dt.float8e4
DR = mybir.MatmulPerfMode.DoubleRow
NPBF = mybir.dt.np(BF16)
EXP = mybir.ActivationFunctionType.Exp
CPY = mybir.ActivationFunctionType.Copy
MUL = mybir.AluOpType.mult
ADD = mybir.AluOpType.add

_last_results = None  # BassKernelResults of the most recent run (for test.py)


@functools.lru_cache(maxsize=4)
def _build(u: int):
    assert u % 128 == 0 and 0 <= u <= 512, f"unsupported unseen_size {u}"
    nc = bacc.Bacc(None, target_bir_lowering=False)
    xT = nc.dram_tensor("xT", [C, N], BF16, kind="ExternalInput")
    wqkT = nc.dram_tensor("wqkT", [C, 6 * D], BF16, kind="ExternalInput")
    wvT = nc.dram_tensor("wvT", [C, NH * D], BF16, kind="ExternalInput")
    wpT = nc.dram_tensor("wpT", [NH * D, C], BF16, kind="ExternalInput")
    yT = nc.dram_tensor("yT", [C, N], BF16, kind="ExternalOutput")

    kfull = N - u
    T = (kfull + 127) // 128           # key tiles
    QT = N // 128                      # 16 query tiles
    ut0 = kfull // 128                 # first unseen query tile
    NCH = N // 512                     # 4 token chunks
    NU = u // 128                      # unseen tiles

    with nc.allow_low_precision(reason="bf16 staging/outputs"), \
         tile.TileContext(nc) as tc:
        with tc.tile_pool(name="persist", bufs=1) as P:
            xt = P.tile([128, CT, N], BF16)
            wqk = P.tile([128, CT, 6 * D], BF16)
            wv = P.tile([128, CT, NH * D], BF16)
            wp01 = P.tile([128, C], BF16)
            wp2 = P.tile([64, C], BF16)
            qk0 = P.tile([128, N], BF16)   # [q0; q1]
            qk1 = P.tile([128, N], BF16)   # [k0; k1]
            qk2 = P.tile([128, N], BF16)   # [q2; k2]
            qk2b = P.tile([128, N], BF16)  # [-; q2] (dma partition shift)
            vsb = P.tile([128, QT, NH, 64], BF16)
            # token-major normalized attention out: heads 0/1 interleaved per
            # query tile; head 2 in the low half of its own tile (high half
            # junk, transposed but never copied out).
            ao01 = P.tile([128, QT, 128], F32)
            ao2 = P.tile([128, QT, 128], F32)
            aoT01 = P.tile([128, N], BF16)
            aoT2 = P.tile([64, N], BF16)
            otile = [P.tile([128, N], BF16, name=f"ot{co}", tag=f"ot{co}")
                     for co in range(CT)]
            ones1 = P.tile([128, 1], BF16)
            ident = P.tile([128, 128], F32)
            idones = P.tile([128, 128], F32)
            prod = [P.tile([128, max(u, 1)], BF16, name=f"pr{h}", tag=f"pr{h}")
                    for h in range(NH)] if u else []
            esb = [P.tile([128, max(NU, 1)], F32, name=f"e{h}", tag=f"e{h}")
                   for h in range(NH)] if u else []

            nc.vector.memset(ones1[:], 1.0)
            nc.vector.memset(idones[:], 1.0)
            nc.gpsimd.affine_select(
                ident[:], idones[:], pattern=[[-1, 128]],
                compare_op=mybir.AluOpType.is_equal, fill=0.0,
                base=0, channel_multiplier=1)

            # loads: SP queue carries the QK critical path (wqk cols 0:256
            # then x chunks 0,1,3); the idle Act queue takes wv + x chunk 2.
            def load_x(ch, eng=None):
                sl = slice(ch * 512, (ch + 1) * 512)
                (eng or nc.sync).dma_start(
                    xt[:, :, sl],
                    xT[:, sl].rearrange("(t p) f -> p t f", p=128))
            nc.sync.dma_start(
                wqk[:, :, 0:256],
                wqkT[:, 0:256].rearrange("(t p) f -> p t f", p=128))
            nc.sync.dma_start(
                xt[:, :, 0:256],
                xT[:, 0:256].rearrange("(t p) f -> p t f", p=128))
            nc.sync.dma_start(
                xt[:, :, 256:512],
                xT[:, 256:512].rearrange("(t p) f -> p t f", p=128))
            nc.sync.dma_start(
                xt[:, :, 512:768],
                xT[:, 512:768].rearrange("(t p) f -> p t f", p=128))
            nc.sync.dma_start(wv[:], wvT.rearrange("(t p) f -> p t f", p=128))
            nc.sync.dma_start(
                xt[:, :, 768:1024],
                xT[:, 768:1024].rearrange("(t p) f -> p t f", p=128))
            load_x(2)
            load_x(3)
            nc.sync.dma_start(
                wqk[:, :, 256:384],
                wqkT[:, 256:384].rearrange("(t p) f -> p t f", p=128))
            nc.sync.dma_start(wp01[:], wpT[0:128, :])
            nc.sync.dma_start(wp2[:], wpT[128:192, :])

            qkv_ps = tc.alloc_tile_pool(name="qkv_ps", bufs=1, space="PSUM")
            st_ps = tc.alloc_tile_pool(name="st_ps", bufs=1, space="PSUM")
            av_ps = tc.alloc_tile_pool(name="av_ps", bufs=1, space="PSUM")
            apool = tc.alloc_tile_pool(name="a_sb", bufs=1)
            scr = tc.alloc_tile_pool(name="scr", bufs=1)

            # ---- filler closures (each emits one psum group of work) ----
            def QKg(ch, fi, half=None):
                def f(state):
                    if half is None:
                        sl = slice(ch * 512, (ch + 1) * 512)
                    else:
                        sl = slice(ch * 512 + half * 256,
                                   ch * 512 + half * 256 + 256)
                    w = 512 if half is None else 256
                    dst = (qk0, qk1)[fi]
                    ps = qkv_ps.tile([128, 512], F32, tag="qkvps", bufs=2)
                    for ct in range(CT):
                        nc.tensor.matmul(
                            ps[:, 0:w], wqk[:, ct, fi * 128:(fi + 1) * 128],
                            xt[:, ct, sl],
                            start=(ct == 0), stop=(ct == CT - 1))
                    nc.vector.tensor_copy(dst[:, sl], ps[:, 0:w])
                return f

            def Vg(nt):
                def f(state):
                    ps = qkv_ps.tile([128, 512], F32, tag="qkvps", bufs=2)
                    for ct in range(CT):
                        nc.tensor.matmul(
                            ps[:, 0:NH * D], xt[:, ct, nt * 128:(nt + 1) * 128],
                            wv[:, ct, :],
                            start=(ct == 0), stop=(ct == CT - 1))
                    nc.vector.tensor_copy(
                        vsb[:, nt, :, :],
                        ps[:, 0:NH * D].rearrange("p (h x) -> p h x", x=64))
                return f

            def Q2g(ch):
                def f(state):
                    sl = slice(ch * 512, (ch + 1) * 512)
                    ps = qkv_ps.tile([128, 512], F32, tag="qkvps", bufs=2)
                    for ct in range(CT):
                        nc.tensor.matmul(
                            ps[:], wqk[:, ct, 256:384], xt[:, ct, sl],
                            start=(ct == 0), stop=(ct == CT - 1))
                    nc.vector.tensor_copy(qk2[:, sl], ps[:])
                return f

            def SHIFT(state):
                nc.sync.dma_start(qk2b[64:128, :], qk2[0:64, :])

            def TP(h, qt):
                """Transpose one token-major ao tile to feature-major.

                Uses a qkvps ring buffer: a transpose is a start=True matmul,
                which pending-zeroes its whole psum bank, so it must not share
                the bank where the L sums accumulate."""
                def f(state):
                    src = ao2 if h == 2 else ao01
                    reg = qkv_ps.tile([128, 512], F32, tag="qkvps", bufs=2)
                    nc.tensor.transpose(reg[:, 0:128], src[:, qt, :], ident[:])
                    if h == 2:
                        nc.vector.tensor_copy(
                            aoT2[:, qt * 128:(qt + 1) * 128], reg[0:64, 0:128])
                    else:
                        nc.vector.tensor_copy(
                            aoT01[:, qt * 128:(qt + 1) * 128], reg[:, 0:128])
                return f

            def PJg(co, ch, act_copy=False):
                def f(state):
                    csl = slice(co * 128, (co + 1) * 128)
                    sl = slice(ch * 512, (ch + 1) * 512)
                    ps = qkv_ps.tile([128, 512], F32, tag="qkvps", bufs=2)
                    nc.tensor.matmul(ps[:], wp01[:, csl], aoT01[:, sl],
                                     start=True, stop=False)
                    nc.tensor.matmul(ps[:], wp2[:, csl], aoT2[:, sl],
                                     start=False, stop=True)
                    if act_copy:
                        nc.scalar.activation(otile[co][:, sl], ps[:], CPY)
                    else:
                        nc.vector.tensor_copy(otile[co][:, sl], ps[:])
                return f

            def alloc_state():
                av = av_ps.tile([128, 8, 64], F32, tag="av", bufs=1)
                # lt: cols 0:8 L sums, 8:12 diag scratch (one psum bank).
                lt = av_ps.tile([128, 16], F32, tag="lt", bufs=1)
                return {"av": av, "lt": lt, "pq": [], "a2": None}

            def emit_diag(h, q_ap, k_ap, base, state):
                """Per-token diagonal q.k for the unseen range, exp'd to esb."""
                lt = state["lt"]
                pr = prod[h][base:base + 64, 0:u]
                nc.vector.tensor_tensor(
                    out=pr, in0=q_ap[:, kfull:N], in1=k_ap[:, kfull:N], op=MUL)
                # start=False: these bytes are still pending-zero from the L
                # bank's group start, so the first write stores cleanly, and
                # we must not wipe the accumulated L columns.
                for j in range(NU):
                    nc.tensor.matmul(
                        lt[:, 8 + j:9 + j],
                        prod[h][base:base + 64, j * 128:(j + 1) * 128],
                        ones1[base:base + 64, :], start=False, stop=False,
                        skip_group_check=True)
                nc.scalar.activation(esb[h][:, 0:NU], lt[:, 8:8 + NU],
                                     EXP, scale=0.125)

            AVLAG = 6

            def emit_av_block(h, state):
                t, a = state["pq"].pop(0)
                av, lt = state["av"], state["lt"]
                # start=True pending-zeroes the whole 2KB psum bank, so only
                # the first matmul touching each bank may set it; later
                # regions' first writes land on pending-zero bytes and store
                # (not accumulate) -- exactly what a fresh group needs.
                for qs in range(8):
                    asub = a[:, qs * 128:(qs + 1) * 128]
                    nc.tensor.matmul(
                        av[:, qs, :], asub, vsb[:, t, h, :],
                        start=(t == 0 and qs == 0),
                        stop=(t == T - 1 and qs == 7),
                        skip_group_check=True)
                    nc.tensor.matmul(
                        lt[:, qs:qs + 1], asub, ones1[:],
                        start=(t == 0 and qs == 0),
                        stop=(t == T - 1 and qs == 7),
                        skip_group_check=True)

            pending = {}

            def finish_pending(part=3, norm_split=False):
                """Flush the previous chunk's lagged AV blocks, its diag
                fix, and its normalization, spread over the next chunk's
                first slots: part=0/1 drain AV blocks, part=2 does diag+norm,
                part=3 does everything remaining."""
                if not pending:
                    return
                h, qc, q_ap, k_ap, base, state = pending["v"]
                if part == 0:
                    for _ in range(len(state["pq"]) // 2):
                        emit_av_block(h, state)
                    return
                if part == 1:
                    while state["pq"]:
                        emit_av_block(h, state)
                    return
                del pending["v"]
                while state["pq"]:
                    emit_av_block(h, state)
                if u and qc == 1:
                    emit_diag(h, q_ap, k_ap, base, state)
                emit_norm(h, qc, state, norm_split)

            def emit_attn_chunk(h, q_ap, k_ap, qc, t_lo, t_hi, state,
                                fillers=None, lag=None):
                """ST+exp for key tiles [t_lo, t_hi); AV lags `lag` steps.

                fillers: dict t -> list of closures drained after ST(t)."""
                if lag is None:
                    lag = AVLAG
                for t in range(t_lo, t_hi):
                    st = st_ps.tile([128, 1024], F32, tag="st", bufs=2)
                    for cc in range(2):
                        qsl = slice(qc * 1024 + cc * 512,
                                    qc * 1024 + cc * 512 + 512)
                        nc.tensor.matmul(
                            st[:, cc * 512:(cc + 1) * 512],
                            k_ap[:, t * 128:t * 128 + 128],
                            q_ap[:, qsl], start=True, stop=True)
                    if t == t_lo:
                        finish_pending(0)
                    elif t == t_lo + 1:
                        finish_pending(3)
                    if fillers:
                        for fn in fillers.get(t, ()):
                            fn(state)
                    if len(state["pq"]) > lag:
                        emit_av_block(h, state)
                    a = apool.tile([128, 1024], BF16, tag="a",
                                   bufs=AVLAG + 2)
                    nc.scalar.activation(a[:], st[:], EXP, scale=0.125)
                    state["pq"].append((t, a))

            def emit_norm(h, qc, state, norm_split=False):
                """Denominator fixup + batched reciprocal + normalization."""
                av, lt = state["av"], state["lt"]
                dsttile = ao2 if h == 2 else ao01
                dstoff = 64 if h == 1 else 0
                for qs in range(8):
                    qt = qc * 8 + qs
                    if qt >= ut0:
                        e = esb[h][:, qt - ut0:qt - ut0 + 1]
                        nc.vector.scalar_tensor_tensor(
                            out=av[:, qs, :], in0=vsb[:, qt, h, :],
                            scalar=e, in1=av[:, qs, :], op0=MUL, op1=ADD)
                u_lo = max(ut0 - qc * 8, 0)
                if u_lo < 8:  # add diag exp into the masked-tile L sums
                    nc.vector.tensor_tensor(
                        out=lt[:, u_lo:8], in0=lt[:, u_lo:8],
                        in1=esb[h][:, qc * 8 + u_lo - ut0:qc * 8 + 8 - ut0],
                        op=ADD)
                rec = scr.tile([128, 8], F32, tag="rec", bufs=2)
                nc.vector.reciprocal(rec[:], lt[:, 0:8])
                for qs in range(8):
                    qt = qc * 8 + qs
                    dst = dsttile[:, qt, dstoff:dstoff + 64]
                    if norm_split and qs % 2:
                        nc.scalar.activation(dst, av[:, qs, :], CPY,
                                             scale=rec[:, qs:qs + 1])
                    else:
                        nc.vector.tensor_scalar_mul(
                            dst, av[:, qs, :], rec[:, qs:qs + 1])

            # ---- filler schedules (tuned for T == 12; fallback: upfront) ----
            fill = {(h, qc): {} for h in range(NH) for qc in range(2)}
            flushf = {(h, qc): [] for h in range(NH) for qc in range(2)}

            def put(h, qc, t, fn):
                fill[(h, qc)].setdefault(t, []).append(fn)

            if T == 12:
                lead = [QKg(0, 0, 0), QKg(0, 0, 1), QKg(0, 1, 0),
                        QKg(0, 1, 1), QKg(1, 0, 0), QKg(1, 0, 1)]
                # x1 halves land around the last two lead groups
                sched = {0: [Vg(0), Vg(1)], 1: [QKg(1, 1), Vg(2)],
                         2: [Vg(3)], 3: [Vg(4)], 4: [Vg(5)], 5: [Vg(6)],
                         6: [Vg(7)], 7: [QKg(2, 1), Vg(8)],
                         8: [QKg(2, 0), Vg(9)], 9: [QKg(3, 0), Vg(10)],
                         10: [Vg(11)]}
                for t, fns in sched.items():
                    for fn in fns:
                        put(0, 0, t, fn)
                put(0, 1, 8, QKg(3, 1))
                for i in range(4):
                    put(0, 1, 2 + i, Vg(12 + i))
                for ch in range(NCH):
                    put(1, 0, 2 + ch, Q2g(ch))
                put(1, 0, 6, SHIFT)
                # transposes read the previous chunk's normalized ao, which
                # lands at slot 1 (finish_pending) -- schedule them from slot 2
                for i in range(8):
                    put(1, 1, 2 + i, TP(0, i))      # pair tiles of qc0
                for i in range(8):
                    put(2, 0, 2 + i, TP(0, 8 + i))  # pair tiles of qc1
                for i in range(8):
                    put(2, 1, 2 + i, TP(2, i))      # head-2 tiles of qc0
                for i in range(CT):
                    put(2, 1, 6 + i, PJg(i, 0))
                put(2, 1, 10, PJg(0, 1))
                put(2, 1, 11, PJg(1, 1))
                pj_tail = None
            else:
                lead = ([QKg(ch, fi) for ch in range(NCH) for fi in (0, 1)]
                        + [Vg(nt) for nt in range(QT)]
                        + [Q2g(ch) for ch in range(NCH)] + [SHIFT])
                pj_tail = [(co, ch, co % 2 == 1)
                           for ch in range(NCH) for co in range(CT)]

            # ---- emission ----
            # dummy matmuls ramp the PE p-state during the initial DMA wait
            # (full clock needs 3us of continuous PE busy); they overlap the
            # x/wqk transfers and abut the first real projection group.
            NDUM = int(os.environ.get("KERNEL_NDUM", "8"))
            if NDUM:
                dmy = qkv_ps.tile([128, 512], F32, tag="qkvps", bufs=2)
                for _ in range(NDUM):
                    nc.tensor.matmul(dmy[0:1, 0:128], idones[0:1, 0:1],
                                     idones[0:1, 0:128], start=True, stop=True)
            for fn in lead:
                fn(None)

            heads = [(qk0[0:64, :], qk1[0:64, :], 0),
                     (qk0[64:128, :], qk1[64:128, :], 64),
                     (qk2b[64:128, :], qk2[64:128, :], 64)]

            for h, (q_ap, k_ap, base) in enumerate(heads):
                for qc in range(2):
                    state = alloc_state()
                    emit_attn_chunk(h, q_ap, k_ap, qc, 0, T, state,
                                    fill[(h, qc)],
                                    lag=3 if (h, qc) == (2, 1) else None)
                    pending["v"] = (h, qc, q_ap, k_ap, base, state)
                    if T != 12:
                        finish_pending()
                        # fallback: transpose finished tiles in place
                        if h == 1:
                            for i in range(8):
                                TP(0, qc * 8 + i)(state)
                        elif h == 2:
                            for i in range(8):
                                TP(2, qc * 8 + i)(state)
            finish_pending(norm_split=True)

            for _pool in (scr, apool, av_ps, st_ps, qkv_ps):
                _pool.release()

            # ---- tail: head-2 qc1 transposes + remaining projection ----
            with tc.tile_pool(name="tp_ps", bufs=1, space="PSUM") as tpps, \
                 tc.tile_pool(name="pj_ps", bufs=1, space="PSUM") as pjps, \
                 tc.tile_pool(name="ost", bufs=1) as ost:
                def tail_tp(qt):
                    reg = tpps.tile([128, 128], F32, tag="tp", bufs=2)
                    nc.tensor.transpose(reg[:], ao2[:, qt, :], ident[:])
                    if qt % 2:
                        nc.scalar.activation(
                            aoT2[:, qt * 128:(qt + 1) * 128], reg[0:64, :], CPY)
                    else:
                        nc.vector.tensor_copy(
                            aoT2[:, qt * 128:(qt + 1) * 128], reg[0:64, :])

                def tail_pj(co, ch, act):
                    csl = slice(co * 128, (co + 1) * 128)
                    sl = slice(ch * 512, (ch + 1) * 512)
                    ps = pjps.tile([128, 512], F32, tag="pj", bufs=3)
                    nc.tensor.matmul(ps[:], wp01[:, csl], aoT01[:, sl],
                                     start=True, stop=False)
                    nc.tensor.matmul(ps[:], wp2[:, csl], aoT2[:, sl],
                                     start=False, stop=True)
                    if act:
                        nc.scalar.activation(otile[co][:, sl], ps[:], CPY)
                    else:
                        nc.vector.tensor_copy(otile[co][:, sl], ps[:])

                if T == 12:
                    for co in range(2, CT):
                        tail_pj(co, 1, co % 2 == 0)
                    for qt in range(8, 16):
                        tail_tp(qt)
                    qeng = [nc.sync, nc.scalar, nc.gpsimd]
                    for co in range(CT):
                        # first half (chunks 0+1) is already staged: drain it
                        # now so the shared DMA device starts early
                        csl = slice(co * 128, (co + 1) * 128)
                        qeng[co % 3].dma_start(yT[csl, 0:1024],
                                               otile[co][:, 0:1024])
                    nalt = 0
                    for co in range(CT):
                        csl = slice(co * 128, (co + 1) * 128)
                        for ch in (2, 3):
                            tail_pj(co, ch, nalt % 2 == 0)
                            sl = slice(ch * 512, (ch + 1) * 512)
                            qeng[nalt % 3].dma_start(yT[csl, sl],
                                                     otile[co][:, sl])
                            nalt += 1
                else:
                    for co, ch, act in pj_tail:
                        tail_pj(co, ch, act)
                    for co in range(CT):
                        csl = slice(co * 128, (co + 1) * 128)
                        nc.sync.dma_start(yT[csl, :], otile[co][:])

    nc.compile()
    return nc


def kernel(**inputs):
    global _last_results
    from concourse.bass_utils import run_bass_kernel_spmd

    x = np.asarray(inputs["x"], np.float32)
    w_qkv = np.asarray(inputs["w_qkv"], np.float32)
    w_proj = np.asarray(inputs["w_proj"], np.float32)
    b_proj = np.asarray(inputs["b_proj"], np.float32)
    u = int(np.asarray(inputs["unseen_size"]))
    B = x.shape[0]

    nc = _build(u)

    wT = np.ascontiguousarray(w_qkv.T).astype(NPBF)        # [768, 2304]
    wpT_full = np.ascontiguousarray(w_proj.T).astype(NPBF)  # [768(ci), 768(co)]
    xTb = [np.ascontiguousarray(x[b].T).astype(NPBF) for b in range(B)]

    in_maps = []
    for core in range(8):
        b, g = divmod(core, 4)
        hs = [3 * g, 3 * g + 1, 3 * g + 2]
        cols = []
        for h in hs[:2]:
            cols += [0 * C + h * D + i for i in range(D)]   # q0 q1
        for h in hs[:2]:
            cols += [1 * C + h * D + i for i in range(D)]   # k0 k1
        cols += [0 * C + hs[2] * D + i for i in range(D)]   # q2
        cols += [1 * C + hs[2] * D + i for i in range(D)]   # k2
        wqkT = np.ascontiguousarray(wT[:, cols])
        vcols = [2 * C + h * D + i for h in hs for i in range(D)]
        wvT = np.ascontiguousarray(wT[:, vcols])
        ci = [h * D + i for h in hs for i in range(D)]
        wpT = np.ascontiguousarray(wpT_full[ci, :])
        in_maps.append({"xT": xTb[b], "wqkT": wqkT, "wvT": wvT, "wpT": wpT})

    trace = bool(int(os.environ.get("KERNEL_TRACE", "0")))
    res = run_bass_kernel_spmd(nc, in_maps, core_ids=list(range(8)), trace=trace)
    _last_results = res

    y = np.zeros((B, N, C), np.float32)
    for core in range(8):
        b = core // 4
        y[b] += np.asarray(res.results[core]["yT"], np.float32).T
    y += b_proj
    return y


# revision 13
# speedup vs baseline: 1.1175x; 1.0223x over previous
"""MCCDecoderAttention Trainium2 kernel (8 NeuronCores), v3.

Sharding: core = b*4 + g  (b in {0,1} batch, g in {0..3} head-group).
Each core computes attention for 3 heads of one batch plus its partial
output projection; the host sums 4 partials per batch and adds b_proj.

Layout (all matmul operands bf16, f32 PSUM accumulation):
  - Q/K projected feature-major into packed 128-row tiles
    qk0=[q0;q1], qk1=[k0;k1], qk2=[q2;k2]; q2 is copied to partitions
    64:128 of qk2b by an SBUF->SBUF DMA so head 2's matmuls have
    matching base partitions.
  - V projected token-major straight into vsb[key, head, 0:64].
  - Scores S^T computed per (head, 1024-query chunk, 128-key tile);
    exp on ScalarE (scale=1/8 folded, no max subtraction needed).
  - AV uses the transposed form: lhsT = A^T subtile [128k x 128q],
    rhs = vsb [128k x 64] -> psum [128q, 64] accumulated over key
    tiles; the softmax denominator L accumulates in parallel via
    1-column ones matmuls.  The narrow free dims halve PE cost vs the
    feature-major AV form.
  - Decoder mask: keys limited to [0, N-u); each unseen query gets its
    diagonal term back via a 1-column ones-matmul of q*k (elementwise),
    exp, and a fused DVE multiply-add into the psum accumulator.
  - Normalized ao is written token-major f32 with heads 0/1 interleaved
    per query tile; PE transposes [128q x 128c] tiles (into spare
    columns of the L psum bank) flip it feature-major for the output
    projection, which contracts [128ci]+[64ci] per psum group.

Scheduling: emission is software-pipelined (ST(t) is emitted before the
AV block of t-1 so the PE never queues behind the exp dependency), and
all projection / transpose / phase-3 work is drained as per-key-tile
"fillers" inside the Act-bound attention loops.
"""

import functools
import os
import sys

for _p in ("/opt/trn_rl_repo", "/root/.axon_site/_ro/trn_rl_repo"):
    if os.path.isdir(_p) and _p not in sys.path:
        sys.path.insert(0, _p)

import numpy as np

import concourse.bacc as bacc
import concourse.tile as tile
from concourse import mybir

N, C, D = 2048, 768, 64
NH = 3            # heads per core
CT = C // 128     # 6 contraction tiles
F32 = mybir.dt.float32
BF16 = mybir.dt.bfloat16
FP8 = mybir.dt.float8e4
DR = mybir.MatmulPerfMode.DoubleRow
NPBF = mybir.dt.np(BF16)
EXP = mybir.ActivationFunctionType.Exp
CPY = mybir.ActivationFunctionType.Copy
MUL = mybir.AluOpType.mult
ADD = mybir.AluOpType.add

_last_results = None  # BassKernelResults of the most recent run (for test.py)


@functools.lru_cache(maxsize=4)
def _build(u: int):
    assert u % 128 == 0 and 0 <= u <= 512, f"unsupported unseen_size {u}"
    nc = bacc.Bacc(None, target_bir_lowering=False)
    xT = nc.dram_tensor("xT", [C, N], BF16, kind="ExternalInput")
    wqkT = nc.dram_tensor("wqkT", [C, 6 * D], BF16, kind="ExternalInput")
    wvT = nc.dram_tensor("wvT", [C, NH * D], BF16, kind="ExternalInput")
    wpT = nc.dram_tensor("wpT", [NH * D, C], BF16, kind="ExternalInput")
    yT = nc.dram_tensor("yT", [C, N], BF16, kind="ExternalOutput")

    kfull = N - u
    T = (kfull + 127) // 128           # key tiles
    QT = N // 128                      # 16 query tiles
    ut0 = kfull // 128                 # first unseen query tile
    NCH = N // 512                     # 4 token chunks
    NU = u // 128                      # unseen tiles

    with nc.allow_low_precision(reason="bf16 staging/outputs"), \
         tile.TileContext(nc) as tc:
        with tc.tile_pool(name="persist", bufs=1) as P:
            xt = P.tile([128, CT, N], BF16)
            wqk = P.tile([128, CT, 6 * D], BF16)
            wv = P.tile([128, CT, NH * D], BF16)
            wp01 = P.tile([128, C], BF16)
            wp2 = P.tile([64, C], BF16)
            qk0 = P.tile([128, N], BF16)   # [q0; q1]
            qk1 = P.tile([128, N], BF16)   # [k0; k1]
            qk2 = P.tile([128, N], BF16)   # [q2; k2]
            qk2b = P.tile([128, N], BF16)  # [-; q2] (dma partition shift)
            vsb = P.tile([128, QT, NH, 64], BF16)
            # token-major normalized attention out: heads 0/1 interleaved per
            # query tile; head 2 in the low half of its own tile (high half
            # junk, transposed but never copied out).
            ao01 = P.tile([128, QT, 128], F32)
            ao2 = P.tile([128, QT, 128], F32)
            aoT01 = P.tile([128, N], BF16)
            aoT2 = P.tile([64, N], BF16)
            otile = [P.tile([128, N], BF16, name=f"ot{co}", tag=f"ot{co}")
                     for co in range(CT)]
            ones1 = P.tile([128, 1], BF16)
            ident = P.tile([128, 128], F32)
            idones = P.tile([128, 128], F32)
            prod = [P.tile([128, max(u, 1)], BF16, name=f"pr{h}", tag=f"pr{h}")
                    for h in range(NH)] if u else []
            esb = [P.tile([128, max(NU, 1)], F32, name=f"e{h}", tag=f"e{h}")
                   for h in range(NH)] if u else []

            nc.vector.memset(ones1[:], 1.0)
            nc.vector.memset(idones[:], 1.0)
            nc.gpsimd.affine_select(
                ident[:], idones[:], pattern=[[-1, 128]],
                compare_op=mybir.AluOpType.is_equal, fill=0.0,
                base=0, channel_multiplier=1)

            # loads: SP queue carries the QK critical path (wqk cols 0:256
            # then x chunks 0,1,3); the idle Act queue takes wv + x chunk 2.
            def load_x(ch, eng=None):
                sl = slice(ch * 512, (ch + 1) * 512)
                (eng or nc.sync).dma_start(
                    xt[:, :, sl],
                    xT[:, sl].rearrange("(t p) f -> p t f", p=128))
            nc.sync.dma_start(
                wqk[:, :, 0:256],
                wqkT[:, 0:256].rearrange("(t p) f -> p t f", p=128))
            nc.sync.dma_start(
                xt[:, :, 0:256],
                xT[:, 0:256].rearrange("(t p) f -> p t f", p=128))
            nc.sync.dma_start(
                xt[:, :, 256:512],
                xT[:, 256:512].rearrange("(t p) f -> p t f", p=128))
            nc.sync.dma_start(
                xt[:, :, 512:768],
                xT[:, 512:768].rearrange("(t p) f -> p t f", p=128))
            nc.sync.dma_start(wv[:], wvT.rearrange("(t p) f -> p t f", p=128))
            nc.sync.dma_start(
                xt[:, :, 768:1024],
                xT[:, 768:1024].rearrange("(t p) f -> p t f", p=128))
            load_x(2)
            load_x(3)
            nc.sync.dma_start(
                wqk[:, :, 256:384],
                wqkT[:, 256:384].rearrange("(t p) f -> p t f", p=128))
            nc.sync.dma_start(wp01[:], wpT[0:128, :])
            nc.sync.dma_start(wp2[:], wpT[128:192, :])

            qkv_ps = tc.alloc_tile_pool(name="qkv_ps", bufs=1, space="PSUM")
            st_ps = tc.alloc_tile_pool(name="st_ps", bufs=1, space="PSUM")
            av_ps = tc.alloc_tile_pool(name="av_ps", bufs=1, space="PSUM")
            apool = tc.alloc_tile_pool(name="a_sb", bufs=1)
            scr = tc.alloc_tile_pool(name="scr", bufs=1)

            # ---- filler closures (each emits one psum group of work) ----
            def QKg(ch, fi, half=None):
                def f(state):
                    if half is None:
                        sl = slice(ch * 512, (ch + 1) * 512)
                    else:
                        sl = slice(ch * 512 + half * 256,
                                   ch * 512 + half * 256 + 256)
                    w = 512 if half is None else 256
                    dst = (qk0, qk1)[fi]
                    ps = qkv_ps.tile([128, 512], F32, tag="qkvps", bufs=2)
                    for ct in range(CT):
                        nc.tensor.matmul(
                            ps[:, 0:w], wqk[:, ct, fi * 128:(fi + 1) * 128],
                            xt[:, ct, sl],
                            start=(ct == 0), stop=(ct == CT - 1))
                    nc.vector.tensor_copy(dst[:, sl], ps[:, 0:w])
                return f

            def Vg(nt):
                def f(state):
                    ps = qkv_ps.tile([128, 512], F32, tag="qkvps", bufs=2)
                    for ct in range(CT):
                        nc.tensor.matmul(
                            ps[:, 0:NH * D], xt[:, ct, nt * 128:(nt + 1) * 128],
                            wv[:, ct, :],
                            start=(ct == 0), stop=(ct == CT - 1))
                    nc.vector.tensor_copy(
                        vsb[:, nt, :, :],
                        ps[:, 0:NH * D].rearrange("p (h x) -> p h x", x=64))
                return f

            def Q2g(ch):
                def f(state):
                    sl = slice(ch * 512, (ch + 1) * 512)
                    ps = qkv_ps.tile([128, 512], F32, tag="qkvps", bufs=2)
                    for ct in range(CT):
                        nc.tensor.matmul(
                            ps[:], wqk[:, ct, 256:384], xt[:, ct, sl],
                            start=(ct == 0), stop=(ct == CT - 1))
                    nc.vector.tensor_copy(qk2[:, sl], ps[:])
                return f

            def SHIFT(state):
                nc.sync.dma_start(qk2b[64:128, :], qk2[0:64, :])

            def TP(h, qt):
                """Transpose one token-major ao tile to feature-major.

                Uses a qkvps ring buffer: a transpose is a start=True matmul,
                which pending-zeroes its whole psum bank, so it must not share
                the bank where the L sums accumulate."""
                def f(state):
                    src = ao2 if h == 2 else ao01
                    reg = qkv_ps.tile([128, 512], F32, tag="qkvps", bufs=2)
                    nc.tensor.transpose(reg[:, 0:128], src[:, qt, :], ident[:])
                    if h == 2:
                        nc.vector.tensor_copy(
                            aoT2[:, qt * 128:(qt + 1) * 128], reg[0:64, 0:128])
                    else:
                        nc.vector.tensor_copy(
                            aoT01[:, qt * 128:(qt + 1) * 128], reg[:, 0:128])
                return f

            def PJg(co, ch, act_copy=False):
                def f(state):
                    csl = slice(co * 128, (co + 1) * 128)
                    sl = slice(ch * 512, (ch + 1) * 512)
                    ps = qkv_ps.tile([128, 512], F32, tag="qkvps", bufs=2)
                    nc.tensor.matmul(ps[:], wp01[:, csl], aoT01[:, sl],
                                     start=True, stop=False)
                    nc.tensor.matmul(ps[:], wp2[:, csl], aoT2[:, sl],
                                     start=False, stop=True)
                    if act_copy:
                        nc.scalar.activation(otile[co][:, sl], ps[:], CPY)
                    else:
                        nc.vector.tensor_copy(otile[co][:, sl], ps[:])
                return f

            def alloc_state():
                av = av_ps.tile([128, 8, 64], F32, tag="av", bufs=1)
                # lt: cols 0:8 L sums, 8:12 diag scratch (one psum bank).
                lt = av_ps.tile([128, 16], F32, tag="lt", bufs=1)
                return {"av": av, "lt": lt, "pq": [], "a2": None}

            def emit_diag(h, q_ap, k_ap, base, state):
                """Per-token diagonal q.k for the unseen range, exp'd to esb."""
                lt = state["lt"]
                pr = prod[h][base:base + 64, 0:u]
                nc.vector.tensor_tensor(
                    out=pr, in0=q_ap[:, kfull:N], in1=k_ap[:, kfull:N], op=MUL)
                # start=False: these bytes are still pending-zero from the L
                # bank's group start, so the first write stores cleanly, and
                # we must not wipe the accumulated L columns.
                for j in range(NU):
                    nc.tensor.matmul(
                        lt[:, 8 + j:9 + j],
                        prod[h][base:base + 64, j * 128:(j + 1) * 128],
                        ones1[base:base + 64, :], start=False, stop=False,
                        skip_group_check=True)
                nc.scalar.activation(esb[h][:, 0:NU], lt[:, 8:8 + NU],
                                     EXP, scale=0.125)

            AVLAG = 6

            def emit_av_block(h, state):
                t, a = state["pq"].pop(0)
                av, lt = state["av"], state["lt"]
                # start=True pending-zeroes the whole 2KB psum bank, so only
                # the first matmul touching each bank may set it; later
                # regions' first writes land on pending-zero bytes and store
                # (not accumulate) -- exactly what a fresh group needs.
                for qs in range(8):
                    asub = a[:, qs * 128:(qs + 1) * 128]
                    nc.tensor.matmul(
                        av[:, qs, :], asub, vsb[:, t, h, :],
                        start=(t == 0 and qs == 0),
                        stop=(t == T - 1 and qs == 7),
                        skip_group_check=True)
                    nc.tensor.matmul(
                        lt[:, qs:qs + 1], asub, ones1[:],
                        start=(t == 0 and qs == 0),
                        stop=(t == T - 1 and qs == 7),
                        skip_group_check=True)

            pending = {}

            def finish_pending(part=3, norm_split=False):
                """Flush the previous chunk's lagged AV blocks, its diag
                fix, and its normalization, spread over the next chunk's
                first slots: part=0/1 drain AV blocks, part=2 does diag+norm,
                part=3 does everything remaining."""
                if not pending:
                    return
                h, qc, q_ap, k_ap, base, state = pending["v"]
                if part == 0:
                    for _ in range(min(4, len(state["pq"]))):
                        emit_av_block(h, state)
                    return
                if part == 1:
                    while state["pq"]:
                        emit_av_block(h, state)
                    return
                del pending["v"]
                while state["pq"]:
                    emit_av_block(h, state)
                if u and qc == 1:
                    emit_diag(h, q_ap, k_ap, base, state)
                emit_norm(h, qc, state, norm_split)

            def emit_attn_chunk(h, q_ap, k_ap, qc, t_lo, t_hi, state,
                                fillers=None, lag=None):
                """ST+exp for key tiles [t_lo, t_hi); AV lags `lag` steps.

                fillers: dict t -> list of closures drained after ST(t)."""
                if lag is None:
                    lag = AVLAG
                for t in range(t_lo, t_hi):
                    st = st_ps.tile([128, 1024], F32, tag="st", bufs=2)
                    for cc in range(2):
                        qsl = slice(qc * 1024 + cc * 512,
                                    qc * 1024 + cc * 512 + 512)
                        nc.tensor.matmul(
                            st[:, cc * 512:(cc + 1) * 512],
                            k_ap[:, t * 128:t * 128 + 128],
                            q_ap[:, qsl], start=True, stop=True)
                    if t == t_lo:
                        finish_pending(0)
                    elif t == t_lo + 1:
                        finish_pending(3)
                    if fillers:
                        for fn in fillers.get(t, ()):
                            fn(state)
                    if len(state["pq"]) > lag:
                        emit_av_block(h, state)
                    a = apool.tile([128, 1024], BF16, tag="a",
                                   bufs=AVLAG + 2)
                    nc.scalar.activation(a[:], st[:], EXP, scale=0.125)
                    state["pq"].append((t, a))

            def emit_norm(h, qc, state, norm_split=False):
                """Denominator fixup + batched reciprocal + normalization."""
                av, lt = state["av"], state["lt"]
                dsttile = ao2 if h == 2 else ao01
                dstoff = 64 if h == 1 else 0
                for qs in range(8):
                    qt = qc * 8 + qs
                    if qt >= ut0:
                        e = esb[h][:, qt - ut0:qt - ut0 + 1]
                        nc.vector.scalar_tensor_tensor(
                            out=av[:, qs, :], in0=vsb[:, qt, h, :],
                            scalar=e, in1=av[:, qs, :], op0=MUL, op1=ADD)
                u_lo = max(ut0 - qc * 8, 0)
                if u_lo < 8:  # add diag exp into the masked-tile L sums
                    nc.vector.tensor_tensor(
                        out=lt[:, u_lo:8], in0=lt[:, u_lo:8],
                        in1=esb[h][:, qc * 8 + u_lo - ut0:qc * 8 + 8 - ut0],
                        op=ADD)
                rec = scr.tile([128, 8], F32, tag="rec", bufs=2)
                nc.vector.reciprocal(rec[:], lt[:, 0:8])
                for qs in range(8):
                    qt = qc * 8 + qs
                    dst = dsttile[:, qt, dstoff:dstoff + 64]
                    if norm_split and qs % 2:
                        nc.scalar.activation(dst, av[:, qs, :], CPY,
                                             scale=rec[:, qs:qs + 1])
                    else:
                        nc.vector.tensor_scalar_mul(
                            dst, av[:, qs, :], rec[:, qs:qs + 1])

            # ---- filler schedules (tuned for T == 12; fallback: upfront) ----
            fill = {(h, qc): {} for h in range(NH) for qc in range(2)}
            flushf = {(h, qc): [] for h in range(NH) for qc in range(2)}

            def put(h, qc, t, fn):
                fill[(h, qc)].setdefault(t, []).append(fn)

            if T == 12:
                lead = [QKg(0, 0, 0), QKg(0, 0, 1), QKg(0, 1, 0),
                        QKg(0, 1, 1), QKg(1, 0, 0), QKg(1, 0, 1)]
                # x1 halves land around the last two lead groups
                sched = {0: [Vg(0), Vg(1)], 1: [QKg(1, 1), Vg(2)],
                         2: [Vg(3)], 3: [Vg(4)], 4: [Vg(5)], 5: [Vg(6)],
                         6: [Vg(7)], 7: [QKg(2, 1), Vg(8)],
                         8: [QKg(2, 0), Vg(9)], 9: [QKg(3, 0), Vg(10)],
                         10: [Vg(11)]}
                for t, fns in sched.items():
                    for fn in fns:
                        put(0, 0, t, fn)
                put(0, 1, 8, QKg(3, 1))
                for i in range(4):
                    put(0, 1, 2 + i, Vg(12 + i))
                for ch in range(NCH):
                    put(1, 0, 4 + ch, Q2g(ch))
                put(1, 0, 8, SHIFT)
                # transposes read the previous chunk's normalized ao, which
                # lands at slot 1 (finish_pending) -- schedule them from slot 2
                for i in range(8):
                    put(1, 1, 2 + i, TP(0, i))      # pair tiles of qc0
                for i in range(8):
                    put(2, 0, 2 + i, TP(0, 8 + i))  # pair tiles of qc1
                for i in range(8):
                    put(2, 1, 2 + i, TP(2, i))      # head-2 tiles of qc0
                for i in range(CT):
                    put(2, 1, 6 + i, PJg(i, 0))
                def PJD(co):
                    def f(state):
                        PJg(co, 1)(state)
                        csl = slice(co * 128, (co + 1) * 128)
                        nc.sync.dma_start(yT[csl, 0:1024],
                                          otile[co][:, 0:1024])
                    return f
                put(2, 1, 10, PJD(0))
                put(2, 1, 11, PJD(1))
                pj_tail = None
            else:
                lead = ([QKg(ch, fi) for ch in range(NCH) for fi in (0, 1)]
                        + [Vg(nt) for nt in range(QT)]
                        + [Q2g(ch) for ch in range(NCH)] + [SHIFT])
                pj_tail = [(co, ch, co % 2 == 1)
                           for ch in range(NCH) for co in range(CT)]

            # ---- emission ----
            # dummy matmuls ramp the PE p-state during the initial DMA wait
            # (full clock needs 3us of continuous PE busy); they overlap the
            # x/wqk transfers and abut the first real projection group.
            NDUM = int(os.environ.get("KERNEL_NDUM", "8"))
            if NDUM:
                dmy = qkv_ps.tile([128, 512], F32, tag="qkvps", bufs=2)
                for _ in range(NDUM):
                    nc.tensor.matmul(dmy[0:1, 0:128], idones[0:1, 0:1],
                                     idones[0:1, 0:128], start=True, stop=True)
            for fn in lead:
                fn(None)

            heads = [(qk0[0:64, :], qk1[0:64, :], 0),
                     (qk0[64:128, :], qk1[64:128, :], 64),
                     (qk2b[64:128, :], qk2[64:128, :], 64)]

            for h, (q_ap, k_ap, base) in enumerate(heads):
                for qc in range(2):
                    state = alloc_state()
                    emit_attn_chunk(h, q_ap, k_ap, qc, 0, T, state,
                                    fill[(h, qc)],
                                    lag=3 if (h, qc) == (2, 1) else None)
                    pending["v"] = (h, qc, q_ap, k_ap, base, state)
                    if T != 12:
                        finish_pending()
                        # fallback: transpose finished tiles in place
                        if h == 1:
                            for i in range(8):
                                TP(0, qc * 8 + i)(state)
                        elif h == 2:
                            for i in range(8):
                                TP(2, qc * 8 + i)(state)
            finish_pending(norm_split=True)

            for _pool in (scr, apool, av_ps, st_ps, qkv_ps):
                _pool.release()

            # ---- tail: head-2 qc1 transposes + remaining projection ----
            with tc.tile_pool(name="tp_ps", bufs=1, space="PSUM") as tpps, \
                 tc.tile_pool(name="pj_ps", bufs=1, space="PSUM") as pjps, \
                 tc.tile_pool(name="ost", bufs=1) as ost:
                def tail_tp(qt):
                    reg = tpps.tile([128, 128], F32, tag="tp", bufs=2)
                    nc.tensor.transpose(reg[:], ao2[:, qt, :], ident[:])
                    if qt % 2 == 0:
                        nc.scalar.activation(
                            aoT2[:, qt * 128:(qt + 1) * 128], reg[0:64, :], CPY)
                    else:
                        nc.vector.tensor_copy(
                            aoT2[:, qt * 128:(qt + 1) * 128], reg[0:64, :])

                def tail_pj(co, ch, act):
                    csl = slice(co * 128, (co + 1) * 128)
                    sl = slice(ch * 512, (ch + 1) * 512)
                    ps = pjps.tile([128, 512], F32, tag="pj", bufs=3)
                    nc.tensor.matmul(ps[:], wp01[:, csl], aoT01[:, sl],
                                     start=True, stop=False)
                    nc.tensor.matmul(ps[:], wp2[:, csl], aoT2[:, sl],
                                     start=False, stop=True)
                    if act:
                        nc.scalar.activation(otile[co][:, sl], ps[:], CPY)
                    else:
                        nc.vector.tensor_copy(otile[co][:, sl], ps[:])

                if T == 12:
                    for co in range(2, CT):
                        tail_pj(co, 1, co % 2 == 0)
                    for qt in range(8, 16):
                        tail_tp(qt)
                    qeng = [nc.sync, nc.scalar, nc.gpsimd]
                    for co in range(2, CT):
                        # first half (chunks 0+1) is already staged: drain it
                        # now so the shared DMA device starts early
                        csl = slice(co * 128, (co + 1) * 128)
                        qeng[co % 3].dma_start(yT[csl, 0:1024],
                                               otile[co][:, 0:1024])
                    nalt = 0
                    for co in range(CT):
                        csl = slice(co * 128, (co + 1) * 128)
                        for ch in (2, 3):
                            tail_pj(co, ch, nalt % 2 == 0)
                            sl = slice(ch * 512, (ch + 1) * 512)
                            qeng[(nalt + 1) % 3].dma_start(yT[csl, sl],
                                                     otile[co][:, sl])
                            nalt += 1
                else:
                    for co, ch, act in pj_tail:
                        tail_pj(co, ch, act)
                    for co in range(CT):
                        csl = slice(co * 128, (co + 1) * 128)
                        nc.sync.dma_start(yT[csl, :], otile[co][:])

    nc.compile()
    return nc


def kernel(**inputs):
    global _last_results
    from concourse.bass_utils import run_bass_kernel_spmd

    x = np.asarray(inputs["x"], np.float32)
    w_qkv = np.asarray(inputs["w_qkv"], np.float32)
    w_proj = np.asarray(inputs["w_proj"], np.float32)
    b_proj = np.asarray(inputs["b_proj"], np.float32)
    u = int(np.asarray(inputs["unseen_size"]))
    B = x.shape[0]

    nc = _build(u)

    wT = np.ascontiguousarray(w_qkv.T).astype(NPBF)        # [768, 2304]
    wpT_full = np.ascontiguousarray(w_proj.T).astype(NPBF)  # [768(ci), 768(co)]
    xTb = [np.ascontiguousarray(x[b].T).astype(NPBF) for b in range(B)]

    in_maps = []
    for core in range(8):
        b, g = divmod(core, 4)
        hs = [3 * g, 3 * g + 1, 3 * g + 2]
        cols = []
        for h in hs[:2]:
            cols += [0 * C + h * D + i for i in range(D)]   # q0 q1
        for h in hs[:2]:
            cols += [1 * C + h * D + i for i in range(D)]   # k0 k1
        cols += [0 * C + hs[2] * D + i for i in range(D)]   # q2
        cols += [1 * C + hs[2] * D + i for i in range(D)]   # k2
        wqkT = np.ascontiguousarray(wT[:, cols])
        vcols = [2 * C + h * D + i for h in hs for i in range(D)]
        wvT = np.ascontiguousarray(wT[:, vcols])
        ci = [h * D + i for h in hs for i in range(D)]
        wpT = np.ascontiguousarray(wpT_full[ci, :])
        in_maps.append({"xT": xTb[b], "wqkT": wqkT, "wvT": wvT, "wpT": wpT})

    trace = bool(int(os.environ.get("KERNEL_TRACE", "0")))
    res = run_bass_kernel_spmd(nc, in_maps, core_ids=list(range(8)), trace=trace)
    _last_results = res

    y = np.zeros((B, N, C), np.float32)
    for core in range(8):
        b = core // 4
        y[b] += np.asarray(res.results[core]["yT"], np.float32).T
    y += b_proj
    return y
